# revision 1
# baseline (speedup 1.0000x reference)
"""GAT (2-layer graph attention network + mean-pool + classifier) on 8 Trainium2
NeuronCores via Bass/Tile.

Strategy (matches the sharding hint):
- Nodes are partitioned across the 8 cores at whole-graph boundaries (batch
  segments stay local), edges follow the destination node's core.
- Per layer, every core computes the full projected-feature table (layer 1 from
  the replicated input x; layer 2 after an AllGather of the layer-1 output),
  writes it to its DRAM, and gathers per-edge source rows with indirect DMA.
- Edge aggregation (segment softmax + weighted sum) is done fully batched with
  a degree-class slot layout: each destination node owns a fixed-width run of
  edge slots in one SBUF partition, so segment reductions become strided
  tensor_reduce ops.
"""

import os
import sys
import numpy as np

sys.path.insert(0, "/opt/trn_rl_repo")

P = 128          # partitions
NG = 256         # graphs
NCLS = 10        # classes
NCORES = 8

# degree classes (widths); capped at 96 so an L2 batch fits SBUF
CLASSES = [1, 2, 3, 4, 5, 6, 7, 8, 10, 12, 14, 16, 18, 20, 22, 24, 26, 28,
           30, 32, 36, 40, 44, 48, 56, 64, 80, 96]
MASK_NEG = -300.0  # unused now


# ----------------------------------------------------------------------------
# host-side preprocessing (numpy only; index/layout work, no model math)
# ----------------------------------------------------------------------------

def _prep(x, edge_index, batch):
    N = x.shape[0]
    F1 = x.shape[1]
    T1 = (N + P - 1) // P          # column-tiles of the L1 table
    NPAD1 = T1 * P

    src = np.concatenate([edge_index[0], np.arange(N, dtype=np.int64)])
    dst = np.concatenate([edge_index[1], np.arange(N, dtype=np.int64)])
    batch = np.asarray(batch)

    # graph -> node range (batch is sorted)
    gstart = np.searchsorted(batch, np.arange(NG), side="left")
    gend = np.searchsorted(batch, np.arange(NG), side="right")
    # graph -> core, balancing node counts, whole graphs per core
    cum = gend.astype(np.float64)
    bounds = [0]
    for c in range(1, NCORES):
        bounds.append(int(np.searchsorted(cum, c * N / NCORES)))
    bounds.append(NG)
    g0 = np.array(bounds[:-1])
    g1 = np.array(bounds[1:])
    n0 = np.where(g0 < NG, gstart[np.minimum(g0, NG - 1)], N)
    n1 = np.where(g1 > 0, gend[np.minimum(g1 - 1, NG - 1)], 0)
    n0[0] = 0
    n1[-1] = N
    gmax = int((g1 - g0).max())

    # per-core edge lists
    order = np.argsort(dst, kind="stable")
    src_s, dst_s = src[order], dst[order]
    core_edges = []
    for c in range(NCORES):
        lo = np.searchsorted(dst_s, n0[c])
        hi = np.searchsorted(dst_s, n1[c])
        core_edges.append((src_s[lo:hi], dst_s[lo:hi] - n0[c]))

    # degree classes, uniform across cores
    cls_arr = np.array(CLASSES)
    counts = np.zeros((NCORES, len(CLASSES)), np.int64)
    degs = []
    for c in range(NCORES):
        nloc = int(n1[c] - n0[c])
        d = np.bincount(core_edges[c][1], minlength=nloc)
        assert d.min() >= 1 and d.max() <= CLASSES[-1], (d.min(), d.max())
        degs.append(d)
        ci = np.searchsorted(cls_arr, d)
        counts[c] = np.bincount(ci, minlength=len(CLASSES))
    G_w = np.maximum.reduce([(counts[c] + P - 1) // P for c in range(NCORES)])
    active = [i for i in range(len(CLASSES)) if counts[:, i].max() > 0]
    col0 = {}
    e0 = {}
    ncol_total = 0
    necol_total = 0
    for i in active:
        col0[i] = ncol_total
        e0[i] = necol_total
        ncol_total += int(G_w[i])
        necol_total += int(G_w[i]) * CLASSES[i]
    assert ncol_total <= P, ncol_total
    S_total = necol_total
    NCOL = P                     # node columns padded to 128
    SHARD = P * NCOL             # L2 table rows per core
    T2 = SHARD * NCORES // P     # column-tiles of the L2 table

    # global node -> (core, p, j) slot mapping
    g_core = np.zeros(N, np.int32)
    g_p = np.zeros(N, np.int32)
    g_j = np.zeros(N, np.int32)

    per_core = []
    for c in range(NCORES):
        nloc = int(n1[c] - n0[c])
        d = degs[c]
        ci = np.searchsorted(cls_arr, d)
        esrc, edst = core_edges[c]
        # order edges by (node, src)
        eorder = np.lexsort((esrc, edst))
        esrc = esrc[eorder]
        edst = edst[eorder]

        idx1 = np.zeros((P, S_total), np.int32)
        idx2 = np.zeros((P, S_total), np.int32)
        mn = np.zeros((P, S_total), np.float32)  # 1.0 = real edge, 0.0 = pad
        slot_node = np.full((P, NCOL), -1, np.int64)
        # per-edge slot coordinates, vectorized per class
        e_p = np.zeros(len(esrc), np.int64)       # partition of each edge slot
        e_col = np.zeros(len(esrc), np.int64)     # free column of each edge slot
        for i in active:
            w = CLASSES[i]
            nodes = np.nonzero(ci == i)[0]
            if len(nodes) == 0:
                continue
            s = np.arange(len(nodes))
            pp = s % P
            jj = col0[i] + s // P
            slot_node[pp, jj] = nodes
            g_core[n0[c] + nodes] = c
            g_p[n0[c] + nodes] = pp
            g_j[n0[c] + nodes] = jj
            emask = ci[edst] == i
            eidx = np.nonzero(emask)[0]           # sorted by (node, src)
            dn = d[nodes]                         # run length per node, node order
            t = np.repeat(s, dn)                  # class-node rank per edge
            # rank within node run:
            starts = np.concatenate([[0], np.cumsum(dn)[:-1]])
            k = np.arange(len(eidx)) - np.repeat(starts, dn)
            e_p[eidx] = pp[t]
            e_col[eidx] = e0[i] + (jj[t] - col0[i]) * w + k
        sl = esrc
        idx1[e_p, e_col] = ((sl % P) * T1 + sl // P).astype(np.int32)
        mn[e_p, e_col] = 1.0
        per_core.append(dict(idx1=idx1, idx2=idx2, mn=mn, slot_node=slot_node,
                             esrc=esrc, e_p=e_p, e_col=e_col, ci=ci, d=d))

    # second pass: idx2 (needs global slot map)
    for c in range(NCORES):
        pc = per_core[c]
        sl = pc["esrc"]
        r2 = g_p[sl].astype(np.int64) * T2 + (NCOL * g_core[sl].astype(np.int64) + g_j[sl])
        pc["idx2"][pc["e_p"], pc["e_col"]] = r2.astype(np.int32)

    # ownrow arrays + wpool + b2g
    host = []
    for c in range(NCORES):
        pc = per_core[c]
        sn = pc["slot_node"]
        valid = sn >= 0
        nidx = np.where(valid, sn, 0)
        ownrow1 = ((nidx + n0[c]) % P * T1 + (nidx + n0[c]) // P).astype(np.int32)
        ownrow1[~valid] = 0
        jgrid = np.broadcast_to(np.arange(NCOL)[None, :], (P, NCOL))
        pgrid = np.broadcast_to(np.arange(P)[:, None], (P, NCOL))
        ownrow2 = (pgrid * T2 + (NCOL * c + jgrid)).astype(np.int32)

        wpool = np.zeros((P, NCOL, gmax), np.float32)
        cnt = gend - gstart
        gnode = batch[np.minimum(nidx + n0[c], N - 1)]  # graph of each slot node
        gl = (gnode - g0[c]).astype(np.int64)
        ok = valid & (gl >= 0) & (gl < gmax)
        w_val = np.where(ok, 1.0 / np.maximum(cnt[np.minimum(gnode, NG - 1)], 1), 0.0)
        ppi, jji = np.nonzero(ok)
        wpool[ppi, jji, gl[ppi, jji]] = w_val[ppi, jji]

        host.append(dict(
            idx1=pc["idx1"], idx2=pc["idx2"], maskneg=pc["mn"],
            ownrow1=ownrow1, ownrow2=ownrow2,
            wpool=wpool.reshape(P, NCOL * gmax),
        ))

    # batch schedules (class, col0_in_class_grid, ncols, ecol0, w)
    def mk_batches(max_slots, max_nodes):
        out = []
        for i in active:
            w = CLASSES[i]
            step = max(1, min(max_nodes, max_slots // w))
            j = 0
            while j < int(G_w[i]):
                nc_ = min(step, int(G_w[i]) - j)
                out.append((CLASSES[i], col0[i] + j, nc_, e0[i] + j * w))
                j += nc_
        return out

    meta = dict(
        N=N, F1=F1, T1=T1, NPAD1=NPAD1, T2=T2, SHARD=SHARD, NCOL=NCOL,
        S_total=S_total, gmax=gmax,
        batches1=mk_batches(128, 32), batches2=mk_batches(96, 24),
        n0=n0.tolist(), n1=n1.tolist(), g0=g0.tolist(), g1=g1.tolist(),
    )
    aux = dict(slot_nodes=[pc["slot_node"] for pc in per_core],
               g_core=g_core, g_p=g_p, g_j=g_j)
    return host, meta, aux


# ----------------------------------------------------------------------------
# program builder
# ----------------------------------------------------------------------------

def build_program(tc, ins, meta):
    import concourse.bass as bass
    import concourse.mybir as mybir
    from concourse.masks import make_identity

    nc = tc.nc
    dt = mybir.dt
    AX = mybir.AxisListType
    OP = mybir.AluOpType
    ACTF = mybir.ActivationFunctionType

    T1, T2 = meta["T1"], meta["T2"]
    NPAD1, SHARD, NCOL = meta["NPAD1"], meta["SHARD"], meta["NCOL"]
    S_total, gmax = meta["S_total"], meta["gmax"]
    F1 = meta["F1"]
    R1, R2 = 80, 130           # table row widths (h + al_s + al_d)

    # DRAM scratch. The gather tables must be plain Internal tensors (offset
    # 0): indirect DMA from arena-offset pool tiles mis-lowers.
    table1 = nc.dram_tensor("table1", [NPAD1, R1], dt.float32, kind="Internal").ap()
    table2 = nc.dram_tensor("table2", [T2 * P, R2], dt.float32, kind="Internal").ap()
    with tc.tile_pool(name="dram", bufs=1, space="DRAM") as dram:
        agi = dram.tile([64, SHARD], dt.float32)
        ago = dram.tile([64 * NCORES, SHARD], dt.float32)

        with tc.tile_pool(name="cst", bufs=1) as cst:
            # ---------------- constants / fused weights ----------------
            # rhs1 is [128, 80] = [W1 | w~s1 | w~d1]
            rhs1 = cst.tile([P, 80], dt.float32)
            w1 = cst.tile([P, 64], dt.float32)
            nc.sync.dma_start(out=w1[:], in_=ins["W1"][:])
            nc.vector.tensor_copy(out=rhs1[:, 0:64], in_=w1[:])
            a1s = cst.tile([P, 64], dt.float32)
            a1d = cst.tile([P, 64], dt.float32)
            nc.sync.dma_start(out=a1s[:], in_=ins["a1s_bc"][:])
            nc.sync.dma_start(out=a1d[:], in_=ins["a1d_bc"][:])
            tmp1 = cst.tile([P, 64], dt.float32)
            nc.vector.tensor_tensor(out=tmp1[:], in0=w1[:], in1=a1s[:], op=OP.mult)
            nc.vector.tensor_reduce(
                out=rhs1[:, 64:72], in_=tmp1[:].rearrange("p (h c) -> p h c", c=8),
                axis=AX.X, op=OP.add)
            nc.vector.tensor_tensor(out=tmp1[:], in0=w1[:], in1=a1d[:], op=OP.mult)
            nc.vector.tensor_reduce(
                out=rhs1[:, 72:80], in_=tmp1[:].rearrange("p (h c) -> p h c", c=8),
                axis=AX.X, op=OP.add)

            rhs2 = cst.tile([64, R2], dt.float32)
            w2 = cst.tile([64, 128], dt.float32)
            nc.sync.dma_start(out=w2[:], in_=ins["W2"][:])
            nc.vector.tensor_copy(out=rhs2[:, 0:128], in_=w2[:])
            a2s = cst.tile([64, 128], dt.float32)
            a2d = cst.tile([64, 128], dt.float32)
            nc.sync.dma_start(out=a2s[:], in_=ins["a2s_bc"][:])
            nc.sync.dma_start(out=a2d[:], in_=ins["a2d_bc"][:])
            tmp2 = cst.tile([64, 128], dt.float32)
            nc.vector.tensor_tensor(out=tmp2[:], in0=w2[:], in1=a2s[:], op=OP.mult)
            nc.vector.tensor_reduce(out=rhs2[:, 128:129], in_=tmp2[:], axis=AX.X, op=OP.add)
            nc.vector.tensor_tensor(out=tmp2[:], in0=w2[:], in1=a2d[:], op=OP.mult)
            nc.vector.tensor_reduce(out=rhs2[:, 129:130], in_=tmp2[:], axis=AX.X, op=OP.add)

            b1bc = cst.tile([P, 64], dt.float32)
            nc.sync.dma_start(out=b1bc[:], in_=ins["b1bc"][:])
            ident = cst.tile([P, P], dt.float32)
            make_identity(nc, ident[:])

            ald1 = cst.tile([P, NCOL * 8], dt.float32)
            ald2 = cst.tile([P, NCOL], dt.float32)
            x1slot = cst.tile([P, NCOL * 64], dt.float32)
            nc.scalar.memzero(x1slot[:])

            # ---------------- P1: L1 dense -> table1 ----------------
            with tc.tile_pool(name="p1", bufs=3) as p1, \
                 tc.tile_pool(name="p1ps", bufs=4, space="PSUM") as p1ps:
                GT = 6
                t = 0
                while t < T1:
                    g = min(GT, T1 - t)
                    xt = p1.tile([P, GT * P], dt.float32, tag="xt")
                    nc.sync.dma_start(out=xt[:, :g * P],
                                      in_=ins["xT"][:, t * P:(t + g) * P])
                    ps = p1ps.tile([P, GT * R1], dt.float32, tag="ps")
                    for i in range(g):
                        nc.tensor.matmul(out=ps[:, i * R1:(i + 1) * R1],
                                         lhsT=xt[:, i * P:(i + 1) * P],
                                         rhs=rhs1[:], start=True, stop=True)
                    st = p1.tile([P, GT * R1], dt.float32, tag="st")
                    nc.vector.tensor_copy(out=st[:, :g * R1], in_=ps[:, :g * R1])
                    nc.sync.dma_start(
                        out=table1[:].rearrange("(p t) r -> p (t r)", p=P)[:, t * R1:(t + g) * R1],
                        in_=st[:, :g * R1])
                    t += g

            # ---------------- P2: gather al_d1 for own nodes ----------------
            own1 = cst.tile([P, NCOL], dt.int32)
            nc.sync.dma_start(out=own1[:], in_=ins["ownrow1"][:])
            with tc.tile_pool(name="p2", bufs=1) as p2:
                tmp = p2.tile([P, NCOL * R1], dt.float32)
                for s in range(NCOL):
                    nc.gpsimd.indirect_dma_start(
                        out=tmp[:, s * R1:(s + 1) * R1], out_offset=None,
                        in_=table1[:],
                        in_offset=bass.IndirectOffsetOnAxis(ap=own1[:, s:s + 1], axis=0))
                nc.vector.tensor_copy(
                    out=ald1[:],
                    in_=tmp[:].rearrange("p (n r) -> p n r", r=R1)[:, :, 72:80])

            # ---------------- P3: L1 edge phase ----------------
            if not os.environ.get("GAT_NOEDGE"):
                _edge_phase(tc, ins, meta, layer=1, table=table1, ald=ald1,
                            out_slot=x1slot, wpool=None, pool_psum=None)

            # bias + relu
            nc.vector.tensor_tensor(
                out=x1slot[:].rearrange("p (n f) -> p n f", f=64),
                in0=x1slot[:].rearrange("p (n f) -> p n f", f=64),
                in1=b1bc[:].rearrange("p (o f) -> p o f", o=1).to_broadcast([P, NCOL, 64]),
                op=OP.add)
            nc.scalar.activation(out=x1slot[:], in_=x1slot[:], func=ACTF.Relu)

            if "dbg_x1" in ins:
                nc.sync.dma_start(out=ins["dbg_x1"][:], in_=x1slot[:])
                nc.sync.dma_start(out=ins["dbg_ald1"][:], in_=ald1[:])
            if os.environ.get("GAT_STOP"):
                return

            # ---------------- P4: transpose + AllGather ----------------
            with tc.tile_pool(name="p4", bufs=1) as p4, \
                 tc.tile_pool(name="p4ps", bufs=4, space="PSUM") as p4ps:
                x1T = p4.tile([64, SHARD], dt.float32)
                for j2 in range(0, NCOL, 2):
                    ps = p4ps.tile([64, 2 * P], dt.float32, tag="tp")
                    for k in range(2):
                        j = j2 + k
                        nc.tensor.transpose(
                            out=ps[:, k * P:(k + 1) * P],
                            in_=x1slot[:, j * 64:(j + 1) * 64], identity=ident[:])
                    nc.vector.tensor_copy(out=x1T[:, j2 * P:(j2 + 2) * P], in_=ps[:])
                nc.sync.dma_start(out=agi[:], in_=x1T[:])
            nc.gpsimd.collective_compute(
                "AllGather", mybir.AluOpType.bypass,
                replica_groups=[list(range(NCORES))],
                ins=[agi[:].opt()], outs=[ago[:].opt()])

            # ---------------- P5: L2 dense -> table2 ----------------
            with tc.tile_pool(name="p5", bufs=3) as p5, \
                 tc.tile_pool(name="p5ps", bufs=4, space="PSUM") as p5ps:
                GL = 8          # tiles per load
                GP = 3          # tiles per psum bank
                for o in range(NCORES):
                    for jl in range(0, NCOL, GL):
                        blk = p5.tile([64, GL * P], dt.float32, tag="blk")
                        nc.sync.dma_start(out=blk[:],
                                          in_=ago[o * 64:(o + 1) * 64, jl * P:(jl + GL) * P])
                        jp = 0
                        while jp < GL:
                            gp = min(GP, GL - jp)
                            ps = p5ps.tile([P, GP * R2], dt.float32, tag="ps2")
                            for i in range(gp):
                                nc.tensor.matmul(
                                    out=ps[:, i * R2:(i + 1) * R2],
                                    lhsT=blk[:, (jp + i) * P:(jp + i + 1) * P],
                                    rhs=rhs2[:], start=True, stop=True)
                            st = p5.tile([P, GP * R2], dt.float32, tag="st2")
                            nc.vector.tensor_copy(out=st[:, :gp * R2], in_=ps[:, :gp * R2])
                            tt = o * NCOL + jl + jp
                            nc.sync.dma_start(
                                out=table2[:].rearrange("(p t) r -> p (t r)", p=P)[:, tt * R2:(tt + gp) * R2],
                                in_=st[:, :gp * R2])
                            jp += gp

            # ---------------- P6: gather al_d2 for own nodes ----------------
            own2 = cst.tile([P, NCOL], dt.int32)
            nc.sync.dma_start(out=own2[:], in_=ins["ownrow2"][:])
            with tc.tile_pool(name="p6", bufs=2) as p6:
                # ownrow2[p, j] = p*T2 + base + j is consecutive in j, which is
                # exactly the HW indirect-DMA semantic: one index per
                # partition, consecutive rows fill the destination.
                CH = NCOL // 2
                for j0 in range(0, NCOL, CH):
                    tmp = p6.tile([P, CH * R2], dt.float32, tag="tmp")
                    nc.gpsimd.indirect_dma_start(
                        out=tmp[:], out_offset=None, in_=table2[:],
                        in_offset=bass.IndirectOffsetOnAxis(
                            ap=own2[:, j0:j0 + 1], axis=0))
                    nc.vector.tensor_copy(
                        out=ald2[:, j0:j0 + CH],
                        in_=tmp[:].rearrange("p (n r) -> p n r", r=R2)[:, :, 129:130])

            # ---------------- P7: L2 edge phase + pooling ----------------
            with tc.tile_pool(name="pool", bufs=1) as poolp, \
                 tc.tile_pool(name="poolps", bufs=1, space="PSUM") as poolps:
                wpool = poolp.tile([P, NCOL * gmax], dt.float32)
                nc.sync.dma_start(out=wpool[:], in_=ins["wpool"][:])
                pool_ps = poolps.tile([gmax, 128], dt.float32)
                _edge_phase(tc, ins, meta, layer=2, table=table2, ald=ald2,
                            out_slot=None, wpool=wpool, pool_psum=pool_ps)

                # ---------------- P8: head ----------------
                pooled = poolp.tile([gmax, 128], dt.float32)
                nc.vector.tensor_copy(out=pooled[:], in_=pool_ps[:])
                b2g = poolp.tile([gmax, 128], dt.float32)
                nc.sync.dma_start(out=b2g[:], in_=ins["b2g"][:])
                nc.vector.tensor_tensor(out=pooled[:], in0=pooled[:], in1=b2g[:], op=OP.add)
                with tc.tile_pool(name="hps", bufs=1, space="PSUM") as hps:
                    pT_ps = hps.tile([P, gmax], dt.float32)
                    nc.tensor.transpose(out=pT_ps[:], in_=pooled[:],
                                        identity=ident[:gmax, :gmax])
                    pT = poolp.tile([P, gmax], dt.float32)
                    nc.vector.tensor_copy(out=pT[:], in_=pT_ps[:])
                    fcw = poolp.tile([P, NCLS], dt.float32)
                    nc.sync.dma_start(out=fcw[:], in_=ins["fcw"][:])
                    lg_ps = hps.tile([gmax, NCLS], dt.float32)
                    nc.tensor.matmul(out=lg_ps[:], lhsT=pT[:], rhs=fcw[:],
                                     start=True, stop=True)
                    lg = poolp.tile([gmax, NCLS], dt.float32)
                    nc.vector.tensor_copy(out=lg[:], in_=lg_ps[:])
                fcb = poolp.tile([gmax, NCLS], dt.float32)
                nc.sync.dma_start(out=fcb[:], in_=ins["fcb_bc"][:])
                nc.vector.tensor_tensor(out=lg[:], in0=lg[:], in1=fcb[:], op=OP.add)
                # log_softmax
                m = poolp.tile([gmax, 1], dt.float32)
                nc.vector.tensor_reduce(out=m[:], in_=lg[:], axis=AX.X, op=OP.max)
                nc.vector.tensor_scalar(out=lg[:], in0=lg[:], scalar1=m[:],
                                        scalar2=None, op0=OP.subtract)
                ex = poolp.tile([gmax, NCLS], dt.float32)
                nc.scalar.activation(out=ex[:], in_=lg[:], func=ACTF.Exp)
                ss = poolp.tile([gmax, 1], dt.float32)
                nc.vector.tensor_reduce(out=ss[:], in_=ex[:], axis=AX.X, op=OP.add)
                nc.scalar.activation(out=ss[:], in_=ss[:], func=ACTF.Ln)
                nc.vector.tensor_scalar(out=lg[:], in0=lg[:], scalar1=ss[:],
                                        scalar2=None, op0=OP.subtract)
                nc.sync.dma_start(out=ins["out"][:], in_=lg[:])


def _edge_phase(tc, ins, meta, layer, table, ald, out_slot, wpool, pool_psum):
    import concourse.bass as bass
    import concourse.mybir as mybir

    nc = tc.nc
    dt = mybir.dt
    AX = mybir.AxisListType
    OP = mybir.AluOpType
    ACTF = mybir.ActivationFunctionType
    gmax = meta["gmax"]
    NCOL = meta["NCOL"]

    if layer == 1:
        R, F, H, SBMAX, NBMAX = 80, 64, 8, 128, 32
        idx_in, HOFF = ins["idx1"], 64
        batches = meta["batches1"]
    else:
        R, F, H, SBMAX, NBMAX = 130, 128, 1, 96, 24
        idx_in, HOFF = ins["idx2"], 128
        batches = meta["batches2"]

    first_pool = [True]

    with tc.tile_pool(name=f"ed{layer}", bufs=3 if layer == 1 else 2) as ep, \
         tc.tile_pool(name=f"eds{layer}", bufs=4) as eps:
        for (w, j0, ncols, ec0) in batches:
            Sb = ncols * w
            idx = eps.tile([P, Sb], dt.int32, tag="idx")
            nc.sync.dma_start(out=idx[:], in_=idx_in[:, ec0:ec0 + Sb])
            mn = eps.tile([P, Sb], dt.float32, tag="mn")
            nc.sync.dma_start(out=mn[:], in_=ins["maskneg"][:, ec0:ec0 + Sb])
            ed = ep.tile([P, Sb * R], dt.float32, tag="ed")
            for s in range(Sb):
                nc.gpsimd.indirect_dma_start(
                    out=ed[:, s * R:(s + 1) * R], out_offset=None, in_=table[:],
                    in_offset=bass.IndirectOffsetOnAxis(ap=idx[:, s:s + 1], axis=0))

            edv = ed[:].rearrange("p (n k r) -> p n k r", k=w, r=R)
            et = eps.tile([P, Sb * H], dt.float32, tag="et")
            etv = et[:].rearrange("p (n k h) -> p n k h", k=w, h=H)
            # e = al_s[src] + al_d[dst]
            aldv = (ald[:].rearrange("p (n o h) -> p n o h", o=1, h=H)
                    [:, j0:j0 + ncols].to_broadcast([P, ncols, w, H]))
            nc.vector.tensor_tensor(out=etv, in0=edv[:, :, :, HOFF:HOFF + H],
                                    in1=aldv, op=OP.add)
            # exp(leaky_relu(e)), then zero padding slots exactly
            # (ACT's Lrelu ignores alpha on HW: leaky = max(x, 0.2x) on DVE)
            lt = eps.tile([P, Sb * H], dt.float32, tag="lt")
            nc.vector.tensor_scalar(out=lt[:], in0=et[:], scalar1=0.2,
                                    scalar2=None, op0=OP.mult)
            nc.vector.tensor_tensor(out=et[:], in0=et[:], in1=lt[:], op=OP.max)
            nc.scalar.activation(out=et[:], in_=et[:], func=ACTF.Exp)
            mnv = (mn[:].rearrange("p (n k o) -> p n k o", k=w, o=1)
                   .to_broadcast([P, ncols, w, H]))
            nc.vector.tensor_tensor(out=etv, in0=etv, in1=mnv, op=OP.mult)
            # s[d] = sum_k exp
            s = eps.tile([P, ncols * H], dt.float32, tag="s")
            sv = s[:]
            nc.vector.tensor_reduce(
                out=sv.rearrange("p (n h) -> p n h", h=H),
                in_=et[:].rearrange("p (n k h) -> p n h k", k=w, h=H),
                axis=AX.X, op=OP.add)
            nc.vector.tensor_scalar(out=sv, in0=sv, scalar1=1e-16, scalar2=None,
                                    op0=OP.add)
            nc.vector.reciprocal(out=sv, in_=sv)
            # WH = h[src] * exp  (in place on the h part of ed)
            hview = ed[:].rearrange("p (n r) -> p n r", r=R)[:, :, 0:F]
            if H == 1:
                exv = (et[:].rearrange("p (n o) -> p n o", o=1)
                       .to_broadcast([P, Sb, F]))
            else:
                exv = (et[:].rearrange("p (n h o) -> p n h o", h=H, o=1)
                       .to_broadcast([P, Sb, H, F // H]))
                hview = (ed[:].rearrange("p (n r) -> p n r", r=R)
                         [:, :, 0:F].rearrange("p n (h c) -> p n h c", h=H))
            nc.vector.tensor_tensor(out=hview, in0=hview, in1=exv, op=OP.mult)
            # out[d] = sum_k WH / s[d]
            if layer == 1:
                ov = (out_slot[:].rearrange("p (n f) -> p n f", f=F)
                      [:, j0:j0 + ncols])
                x2b = None
            else:
                x2b = ep.tile([P, ncols * F], dt.float32, tag="x2b")
                ov = x2b[:].rearrange("p (n f) -> p n f", f=F)
            nc.vector.tensor_reduce(
                out=ov,
                in_=ed[:].rearrange("p (n k r) -> p n r k", k=w, r=R)[:, :, 0:F, :],
                axis=AX.X, op=OP.add)
            if H == 1:
                sinvv = (s[:].rearrange("p (n o) -> p n o", o=1)
                         .to_broadcast([P, ncols, F]))
                ovv = ov
            else:
                sinvv = (s[:].rearrange("p (n h o) -> p n h o", h=H, o=1)
                         .to_broadcast([P, ncols, H, F // H]))
                ovv = ov.rearrange("p n (h c) -> p n h c", h=H)
            nc.vector.tensor_tensor(out=ovv, in0=ovv, in1=sinvv, op=OP.mult)

            if layer == 2:
                for jj in range(ncols):
                    nc.tensor.matmul(
                        out=pool_psum[:],
                        lhsT=wpool[:, (j0 + jj) * gmax:(j0 + jj + 1) * gmax],
                        rhs=x2b[:, jj * F:(jj + 1) * F],
                        start=first_pool[0],
                        stop=(j0 + jj == NCOL - 1) or
                             ((w, j0, ncols, ec0) == batches[-1] and jj == ncols - 1),
                        skip_group_check=True)
                    first_pool[0] = False


# ----------------------------------------------------------------------------
# runner
# ----------------------------------------------------------------------------

_CACHE = {}


def _get_nc(meta, shapes):
    key = str(sorted(meta.items(), key=lambda kv: kv[0]))
    if key in _CACHE:
        return _CACHE[key]
    import concourse.bacc as bacc
    import concourse.tile as tile
    import concourse.mybir as mybir
    dt = mybir.dt
    nc = bacc.Bacc("TRN2", target_bir_lowering=False, debug=False,
                   num_devices=NCORES)
    dts = {"idx1": dt.int32, "idx2": dt.int32, "ownrow1": dt.int32,
           "ownrow2": dt.int32}
    ins = {}
    for name, shape in shapes.items():
        ins[name] = nc.dram_tensor(name, list(shape),
                                   dts.get(name, dt.float32),
                                   kind="ExternalInput").ap()
    ins["out"] = nc.dram_tensor("out", [meta["gmax"], NCLS], dt.float32,
                                kind="ExternalOutput").ap()
    if os.environ.get("GAT_DEBUG"):
        ins["dbg_x1"] = nc.dram_tensor("dbg_x1", [P, meta["NCOL"] * 64],
                                       dt.float32, kind="ExternalOutput").ap()
        ins["dbg_ald1"] = nc.dram_tensor("dbg_ald1", [P, meta["NCOL"] * 8],
                                         dt.float32, kind="ExternalOutput").ap()
    with tile.TileContext(nc) as tc:
        build_program(tc, ins, meta)
    nc.compile()
    _CACHE[key] = nc
    return nc


def make_inputs(x, edge_index, batch, W1, a_src1, a_dst1, b1, W2, a_src2,
                a_dst2, b2, fc_w, fc_b):
    x = np.asarray(x, np.float32)
    host, meta, aux = _prep(x, np.asarray(edge_index), np.asarray(batch))
    NPAD1 = meta["NPAD1"]
    xT = np.zeros((P, NPAD1), np.float32)
    xT[:, :meta["N"]] = np.ascontiguousarray(x.T)
    shared = dict(
        xT=xT,
        W1=np.asarray(W1, np.float32),
        a1s_bc=np.tile(np.asarray(a_src1, np.float32).reshape(1, 64), (P, 1)),
        a1d_bc=np.tile(np.asarray(a_dst1, np.float32).reshape(1, 64), (P, 1)),
        b1bc=np.tile(np.asarray(b1, np.float32).reshape(1, 64), (P, 1)),
        W2=np.asarray(W2, np.float32),
        a2s_bc=np.tile(np.asarray(a_src2, np.float32).reshape(1, 128), (64, 1)),
        a2d_bc=np.tile(np.asarray(a_dst2, np.float32).reshape(1, 128), (64, 1)),
        fcw=np.asarray(fc_w, np.float32),
        fcb_bc=np.tile(np.asarray(fc_b, np.float32).reshape(1, NCLS),
                       (meta["gmax"], 1)),
    )
    in_maps = []
    for c in range(NCORES):
        m = dict(shared)
        m.update(host[c])
        # b2 contribution per graph (zero for empty graphs)
        g0, g1 = meta["g0"][c], meta["g1"][c]
        nonempty = np.zeros((meta["gmax"], 1), np.float32)
        ge = np.searchsorted(np.asarray(batch), np.arange(NG), side="left")
        gEnd = np.searchsorted(np.asarray(batch), np.arange(NG), side="right")
        cnt = (gEnd - ge)[g0:g1]
        nonempty[:g1 - g0, 0] = (cnt > 0).astype(np.float32)
        m["b2g"] = nonempty * np.asarray(b2, np.float32).reshape(1, 128)
        in_maps.append(m)
    return in_maps, meta, aux


def kernel(x, edge_index, batch, W1, a_src1, a_dst1, b1, W2, a_src2, a_dst2,
           b2, fc_w, fc_b):
    in_maps, meta, aux = make_inputs(x, edge_index, batch, W1, a_src1, a_dst1,
                                     b1, W2, a_src2, a_dst2, b2, fc_w, fc_b)
    global _LAST
    _LAST = dict(meta=meta, aux=aux)
    shapes = {k: v.shape for k, v in in_maps[0].items()}
    nc = _get_nc(meta, shapes)
    from concourse.bass_utils import run_bass_kernel_spmd
    res = run_bass_kernel_spmd(nc, in_maps, core_ids=list(range(NCORES)))
    _LAST["res"] = res
    out = np.zeros((NG, NCLS), np.float32)
    for c in range(NCORES):
        g0, g1 = meta["g0"][c], meta["g1"][c]
        out[g0:g1] = res.results[c]["out"][:g1 - g0]
    return out



# revision 4
# speedup vs baseline: 1.0576x; 1.0576x over previous
"""GAT (2-layer graph attention network + mean-pool + classifier) on 8 Trainium2
NeuronCores via Bass/Tile.

Strategy (matches the sharding hint):
- Nodes are partitioned across the 8 cores at whole-graph boundaries (batch
  segments stay local), edges follow the destination node's core.
- Per layer, every core computes the full projected-feature table (layer 1 from
  the replicated input x; layer 2 after an AllGather of the layer-1 output),
  writes it to its DRAM, and gathers per-edge source rows with indirect DMA.
- Edge aggregation (segment softmax + weighted sum) is done fully batched with
  a degree-class slot layout: each destination node owns a fixed-width run of
  edge slots in one SBUF partition, so segment reductions become strided
  tensor_reduce ops.
"""

import os
import sys
import numpy as np

sys.path.insert(0, "/opt/trn_rl_repo")

P = 128          # partitions
NG = 256         # graphs
NCLS = 10        # classes
NCORES = 8

# degree classes (widths); capped at 96 so an L2 batch fits SBUF
CLASSES = [1, 2, 3, 4, 5, 6, 7, 8, 10, 12, 14, 16, 18, 20, 22, 24, 26, 28,
           30, 32, 36, 40, 44, 48, 56, 64, 80, 96]
MASK_NEG = -300.0  # unused now


# ----------------------------------------------------------------------------
# host-side preprocessing (numpy only; index/layout work, no model math)
# ----------------------------------------------------------------------------

def _prep(x, edge_index, batch):
    N = x.shape[0]
    F1 = x.shape[1]
    T1 = (N + P - 1) // P          # column-tiles of the L1 table
    NPAD1 = T1 * P

    src = np.concatenate([edge_index[0], np.arange(N, dtype=np.int64)])
    dst = np.concatenate([edge_index[1], np.arange(N, dtype=np.int64)])
    batch = np.asarray(batch)

    # graph -> node range (batch is sorted)
    gstart = np.searchsorted(batch, np.arange(NG), side="left")
    gend = np.searchsorted(batch, np.arange(NG), side="right")
    # graph -> core, balancing node counts, whole graphs per core
    cum = gend.astype(np.float64)
    bounds = [0]
    for c in range(1, NCORES):
        bounds.append(int(np.searchsorted(cum, c * N / NCORES)))
    bounds.append(NG)
    g0 = np.array(bounds[:-1])
    g1 = np.array(bounds[1:])
    n0 = np.where(g0 < NG, gstart[np.minimum(g0, NG - 1)], N)
    n1 = np.where(g1 > 0, gend[np.minimum(g1 - 1, NG - 1)], 0)
    n0[0] = 0
    n1[-1] = N
    gmax = int((g1 - g0).max())

    # per-core edge lists
    order = np.argsort(dst, kind="stable")
    src_s, dst_s = src[order], dst[order]
    core_edges = []
    for c in range(NCORES):
        lo = np.searchsorted(dst_s, n0[c])
        hi = np.searchsorted(dst_s, n1[c])
        core_edges.append((src_s[lo:hi], dst_s[lo:hi] - n0[c]))

    # degree classes, uniform across cores
    cls_arr = np.array(CLASSES)
    counts = np.zeros((NCORES, len(CLASSES)), np.int64)
    degs = []
    for c in range(NCORES):
        nloc = int(n1[c] - n0[c])
        d = np.bincount(core_edges[c][1], minlength=nloc)
        assert d.min() >= 1 and d.max() <= CLASSES[-1], (d.min(), d.max())
        degs.append(d)
        ci = np.searchsorted(cls_arr, d)
        counts[c] = np.bincount(ci, minlength=len(CLASSES))
    G_w = np.maximum.reduce([(counts[c] + P - 1) // P for c in range(NCORES)])
    active = [i for i in range(len(CLASSES)) if counts[:, i].max() > 0]
    col0 = {}
    e0 = {}
    ncol_total = 0
    necol_total = 0
    for i in active:
        col0[i] = ncol_total
        e0[i] = necol_total
        ncol_total += int(G_w[i])
        necol_total += int(G_w[i]) * CLASSES[i]
    assert ncol_total <= P, ncol_total
    S_total = necol_total
    NCOL = P                     # node columns padded to 128
    SHARD = P * NCOL             # L2 table rows per core
    T2 = SHARD * NCORES // P     # column-tiles of the L2 table

    # global node -> (core, p, j) slot mapping
    g_core = np.zeros(N, np.int32)
    g_p = np.zeros(N, np.int32)
    g_j = np.zeros(N, np.int32)

    per_core = []
    for c in range(NCORES):
        nloc = int(n1[c] - n0[c])
        d = degs[c]
        ci = np.searchsorted(cls_arr, d)
        esrc, edst = core_edges[c]
        # order edges by (node, src)
        eorder = np.lexsort((esrc, edst))
        esrc = esrc[eorder]
        edst = edst[eorder]

        idx1 = np.zeros((P, S_total), np.int32)
        idx2 = np.zeros((P, S_total), np.int32)
        mn = np.zeros((P, S_total), np.float32)  # 1.0 = real edge, 0.0 = pad
        slot_node = np.full((P, NCOL), -1, np.int64)
        # per-edge slot coordinates, vectorized per class
        e_p = np.zeros(len(esrc), np.int64)       # partition of each edge slot
        e_col = np.zeros(len(esrc), np.int64)     # free column of each edge slot
        for i in active:
            w = CLASSES[i]
            nodes = np.nonzero(ci == i)[0]
            if len(nodes) == 0:
                continue
            s = np.arange(len(nodes))
            pp = s % P
            jj = col0[i] + s // P
            slot_node[pp, jj] = nodes
            g_core[n0[c] + nodes] = c
            g_p[n0[c] + nodes] = pp
            g_j[n0[c] + nodes] = jj
            emask = ci[edst] == i
            eidx = np.nonzero(emask)[0]           # sorted by (node, src)
            dn = d[nodes]                         # run length per node, node order
            t = np.repeat(s, dn)                  # class-node rank per edge
            # rank within node run:
            starts = np.concatenate([[0], np.cumsum(dn)[:-1]])
            k = np.arange(len(eidx)) - np.repeat(starts, dn)
            e_p[eidx] = pp[t]
            e_col[eidx] = e0[i] + (jj[t] - col0[i]) * w + k
        sl = esrc
        idx1[e_p, e_col] = ((sl % P) * T1 + sl // P).astype(np.int32)
        mn[e_p, e_col] = 1.0
        per_core.append(dict(idx1=idx1, idx2=idx2, mn=mn, slot_node=slot_node,
                             esrc=esrc, e_p=e_p, e_col=e_col, ci=ci, d=d))

    # second pass: idx2 (needs global slot map)
    for c in range(NCORES):
        pc = per_core[c]
        sl = pc["esrc"]
        r2 = g_p[sl].astype(np.int64) * T2 + (NCOL * g_core[sl].astype(np.int64) + g_j[sl])
        pc["idx2"][pc["e_p"], pc["e_col"]] = r2.astype(np.int32)

    # ownrow arrays + wpool + b2g
    host = []
    for c in range(NCORES):
        pc = per_core[c]
        sn = pc["slot_node"]
        valid = sn >= 0
        nidx = np.where(valid, sn, 0)
        ownrow1 = ((nidx + n0[c]) % P * T1 + (nidx + n0[c]) // P).astype(np.int32)
        ownrow1[~valid] = 0
        jgrid = np.broadcast_to(np.arange(NCOL)[None, :], (P, NCOL))
        pgrid = np.broadcast_to(np.arange(P)[:, None], (P, NCOL))
        ownrow2 = (pgrid * T2 + (NCOL * c + jgrid)).astype(np.int32)

        wpool = np.zeros((P, NCOL, gmax), np.float32)
        cnt = gend - gstart
        gnode = batch[np.minimum(nidx + n0[c], N - 1)]  # graph of each slot node
        gl = (gnode - g0[c]).astype(np.int64)
        ok = valid & (gl >= 0) & (gl < gmax)
        w_val = np.where(ok, 1.0 / np.maximum(cnt[np.minimum(gnode, NG - 1)], 1), 0.0)
        ppi, jji = np.nonzero(ok)
        wpool[ppi, jji, gl[ppi, jji]] = w_val[ppi, jji]

        host.append(dict(
            idx1=pc["idx1"], idx2=pc["idx2"], maskneg=pc["mn"],
            ownrow1=ownrow1, ownrow2=ownrow2,
            wpool=wpool.reshape(P, NCOL * gmax),
        ))

    # batch schedules (class, col0_in_class_grid, ncols, ecol0, w)
    def mk_batches(max_slots, max_nodes):
        out = []
        for i in active:
            w = CLASSES[i]
            step = max(1, min(max_nodes, max_slots // w))
            j = 0
            while j < int(G_w[i]):
                nc_ = min(step, int(G_w[i]) - j)
                out.append((CLASSES[i], col0[i] + j, nc_, e0[i] + j * w))
                j += nc_
        return out

    meta = dict(
        N=N, F1=F1, T1=T1, NPAD1=NPAD1, T2=T2, SHARD=SHARD, NCOL=NCOL,
        S_total=S_total, gmax=gmax,
        batches1=mk_batches(128, 32), batches2=mk_batches(96, 24),
        n0=n0.tolist(), n1=n1.tolist(), g0=g0.tolist(), g1=g1.tolist(),
    )
    aux = dict(slot_nodes=[pc["slot_node"] for pc in per_core],
               g_core=g_core, g_p=g_p, g_j=g_j)
    return host, meta, aux


# ----------------------------------------------------------------------------
# program builder
# ----------------------------------------------------------------------------

def build_program(tc, ins, meta):
    import concourse.bass as bass
    import concourse.mybir as mybir
    from concourse.masks import make_identity

    nc = tc.nc
    dt = mybir.dt
    AX = mybir.AxisListType
    OP = mybir.AluOpType
    ACTF = mybir.ActivationFunctionType

    T1, T2 = meta["T1"], meta["T2"]
    NPAD1, SHARD, NCOL = meta["NPAD1"], meta["SHARD"], meta["NCOL"]
    S_total, gmax = meta["S_total"], meta["gmax"]
    F1 = meta["F1"]
    R1, R2 = 80, 130           # table row widths (h + al_s + al_d)

    # DRAM scratch. The gather tables must be plain Internal tensors (offset
    # 0): indirect DMA from arena-offset pool tiles mis-lowers.
    table1 = nc.dram_tensor("table1", [NPAD1, R1], dt.float32, kind="Internal").ap()
    table2 = nc.dram_tensor("table2", [T2 * P, R2], dt.float32, kind="Internal").ap()
    with tc.tile_pool(name="dram", bufs=1, space="DRAM") as dram:
        agi = dram.tile([64, SHARD], dt.float32)
        ago = dram.tile([64 * NCORES, SHARD], dt.float32)

        with tc.tile_pool(name="cst", bufs=1) as cst:
            # ---------------- constants / fused weights ----------------
            # rhs1 is [128, 80] = [W1 | w~s1 | w~d1]
            rhs1 = cst.tile([P, 80], dt.float32)
            w1 = cst.tile([P, 64], dt.float32)
            nc.sync.dma_start(out=w1[:], in_=ins["W1"][:])
            nc.vector.tensor_copy(out=rhs1[:, 0:64], in_=w1[:])
            a1s = cst.tile([P, 64], dt.float32)
            a1d = cst.tile([P, 64], dt.float32)
            nc.sync.dma_start(out=a1s[:], in_=ins["a1s_bc"][:])
            nc.sync.dma_start(out=a1d[:], in_=ins["a1d_bc"][:])
            tmp1 = cst.tile([P, 64], dt.float32)
            nc.vector.tensor_tensor(out=tmp1[:], in0=w1[:], in1=a1s[:], op=OP.mult)
            nc.vector.tensor_reduce(
                out=rhs1[:, 64:72], in_=tmp1[:].rearrange("p (h c) -> p h c", c=8),
                axis=AX.X, op=OP.add)
            nc.vector.tensor_tensor(out=tmp1[:], in0=w1[:], in1=a1d[:], op=OP.mult)
            nc.vector.tensor_reduce(
                out=rhs1[:, 72:80], in_=tmp1[:].rearrange("p (h c) -> p h c", c=8),
                axis=AX.X, op=OP.add)

            rhs2 = cst.tile([64, R2], dt.float32)
            w2 = cst.tile([64, 128], dt.float32)
            nc.sync.dma_start(out=w2[:], in_=ins["W2"][:])
            nc.vector.tensor_copy(out=rhs2[:, 0:128], in_=w2[:])
            a2s = cst.tile([64, 128], dt.float32)
            a2d = cst.tile([64, 128], dt.float32)
            nc.sync.dma_start(out=a2s[:], in_=ins["a2s_bc"][:])
            nc.sync.dma_start(out=a2d[:], in_=ins["a2d_bc"][:])
            tmp2 = cst.tile([64, 128], dt.float32)
            nc.vector.tensor_tensor(out=tmp2[:], in0=w2[:], in1=a2s[:], op=OP.mult)
            nc.vector.tensor_reduce(out=rhs2[:, 128:129], in_=tmp2[:], axis=AX.X, op=OP.add)
            nc.vector.tensor_tensor(out=tmp2[:], in0=w2[:], in1=a2d[:], op=OP.mult)
            nc.vector.tensor_reduce(out=rhs2[:, 129:130], in_=tmp2[:], axis=AX.X, op=OP.add)

            b1bc = cst.tile([P, 64], dt.float32)
            nc.sync.dma_start(out=b1bc[:], in_=ins["b1bc"][:])
            ident = cst.tile([P, P], dt.float32)
            make_identity(nc, ident[:])

            ald1 = cst.tile([P, NCOL * 8], dt.float32)
            ald2 = cst.tile([P, NCOL], dt.float32)
            x1slot = cst.tile([P, NCOL * 64], dt.float32)
            nc.scalar.memzero(x1slot[:])

            # ---------------- P1: L1 dense -> table1 ----------------
            with tc.tile_pool(name="p1", bufs=3) as p1, \
                 tc.tile_pool(name="p1ps", bufs=4, space="PSUM") as p1ps:
                GT = 6
                t = 0
                while t < T1:
                    g = min(GT, T1 - t)
                    xt = p1.tile([P, GT * P], dt.float32, tag="xt")
                    nc.sync.dma_start(out=xt[:, :g * P],
                                      in_=ins["xT"][:, t * P:(t + g) * P])
                    ps = p1ps.tile([P, GT * R1], dt.float32, tag="ps")
                    for i in range(g):
                        nc.tensor.matmul(out=ps[:, i * R1:(i + 1) * R1],
                                         lhsT=xt[:, i * P:(i + 1) * P],
                                         rhs=rhs1[:], start=True, stop=True)
                    st = p1.tile([P, GT * R1], dt.float32, tag="st")
                    nc.vector.tensor_copy(out=st[:, :g * R1], in_=ps[:, :g * R1])
                    nc.sync.dma_start(
                        out=table1[:].rearrange("(p t) r -> p (t r)", p=P)[:, t * R1:(t + g) * R1],
                        in_=st[:, :g * R1])
                    t += g

            # ---------------- P2: gather al_d1 for own nodes ----------------
            own1 = cst.tile([P, NCOL], dt.int32)
            nc.sync.dma_start(out=own1[:], in_=ins["ownrow1"][:])
            with tc.tile_pool(name="p2", bufs=1) as p2:
                tmp = p2.tile([P, NCOL * R1], dt.float32)
                for s in range(NCOL):
                    nc.gpsimd.indirect_dma_start(
                        out=tmp[:, s * R1:(s + 1) * R1], out_offset=None,
                        in_=table1[:],
                        in_offset=bass.IndirectOffsetOnAxis(ap=own1[:, s:s + 1], axis=0))
                nc.vector.tensor_copy(
                    out=ald1[:],
                    in_=tmp[:].rearrange("p (n r) -> p n r", r=R1)[:, :, 72:80])

            # ---------------- P3: L1 edge phase ----------------
            if not os.environ.get("GAT_NOEDGE"):
                _edge_phase(tc, ins, meta, layer=1, table=table1, ald=ald1,
                            out_slot=x1slot, wpool=None, pool_psum=None)

            # bias + relu
            nc.vector.tensor_tensor(
                out=x1slot[:].rearrange("p (n f) -> p n f", f=64),
                in0=x1slot[:].rearrange("p (n f) -> p n f", f=64),
                in1=b1bc[:].rearrange("p (o f) -> p o f", o=1).to_broadcast([P, NCOL, 64]),
                op=OP.add)
            nc.scalar.activation(out=x1slot[:], in_=x1slot[:], func=ACTF.Relu)

            if "dbg_x1" in ins:
                nc.sync.dma_start(out=ins["dbg_x1"][:], in_=x1slot[:])
                nc.sync.dma_start(out=ins["dbg_ald1"][:], in_=ald1[:])
            if os.environ.get("GAT_STOP"):
                return

            # ---------------- P4: transpose + AllGather ----------------
            with tc.tile_pool(name="p4", bufs=1) as p4, \
                 tc.tile_pool(name="p4ps", bufs=4, space="PSUM") as p4ps:
                x1T = p4.tile([64, SHARD], dt.float32)
                for j2 in range(0, NCOL, 2):
                    ps = p4ps.tile([64, 2 * P], dt.float32, tag="tp")
                    for k in range(2):
                        j = j2 + k
                        nc.tensor.transpose(
                            out=ps[:, k * P:(k + 1) * P],
                            in_=x1slot[:, j * 64:(j + 1) * 64], identity=ident[:])
                    nc.vector.tensor_copy(out=x1T[:, j2 * P:(j2 + 2) * P], in_=ps[:])
                nc.sync.dma_start(out=agi[:], in_=x1T[:])
            nc.gpsimd.collective_compute(
                "AllGather", mybir.AluOpType.bypass,
                replica_groups=[list(range(NCORES))],
                ins=[agi[:].opt()], outs=[ago[:].opt()])

            # ---------------- P5: L2 dense -> table2 ----------------
            with tc.tile_pool(name="p5", bufs=3) as p5, \
                 tc.tile_pool(name="p5ps", bufs=4, space="PSUM") as p5ps:
                GL = 8          # tiles per load
                GP = 3          # tiles per psum bank
                for o in range(NCORES):
                    for jl in range(0, NCOL, GL):
                        blk = p5.tile([64, GL * P], dt.float32, tag="blk")
                        nc.sync.dma_start(out=blk[:],
                                          in_=ago[o * 64:(o + 1) * 64, jl * P:(jl + GL) * P])
                        jp = 0
                        while jp < GL:
                            gp = min(GP, GL - jp)
                            ps = p5ps.tile([P, GP * R2], dt.float32, tag="ps2")
                            for i in range(gp):
                                nc.tensor.matmul(
                                    out=ps[:, i * R2:(i + 1) * R2],
                                    lhsT=blk[:, (jp + i) * P:(jp + i + 1) * P],
                                    rhs=rhs2[:], start=True, stop=True)
                            st = p5.tile([P, GP * R2], dt.float32, tag="st2")
                            nc.vector.tensor_copy(out=st[:, :gp * R2], in_=ps[:, :gp * R2])
                            tt = o * NCOL + jl + jp
                            nc.sync.dma_start(
                                out=table2[:].rearrange("(p t) r -> p (t r)", p=P)[:, tt * R2:(tt + gp) * R2],
                                in_=st[:, :gp * R2])
                            jp += gp

            # ---------------- P6: gather al_d2 for own nodes ----------------
            own2 = cst.tile([P, NCOL], dt.int32)
            nc.sync.dma_start(out=own2[:], in_=ins["ownrow2"][:])
            with tc.tile_pool(name="p6", bufs=2) as p6:
                # ownrow2[p, j] = p*T2 + base + j is consecutive in j, which is
                # exactly the HW indirect-DMA semantic: one index per
                # partition, consecutive rows fill the destination.
                CH = NCOL // 2
                for j0 in range(0, NCOL, CH):
                    tmp = p6.tile([P, CH * R2], dt.float32, tag="tmp")
                    nc.gpsimd.indirect_dma_start(
                        out=tmp[:], out_offset=None, in_=table2[:],
                        in_offset=bass.IndirectOffsetOnAxis(
                            ap=own2[:, j0:j0 + 1], axis=0))
                    nc.vector.tensor_copy(
                        out=ald2[:, j0:j0 + CH],
                        in_=tmp[:].rearrange("p (n r) -> p n r", r=R2)[:, :, 129:130])

            # ---------------- P7: L2 edge phase + pooling ----------------
            with tc.tile_pool(name="pool", bufs=1) as poolp, \
                 tc.tile_pool(name="poolps", bufs=1, space="PSUM") as poolps:
                wpool = poolp.tile([P, NCOL * gmax], dt.float32)
                nc.sync.dma_start(out=wpool[:], in_=ins["wpool"][:])
                pool_ps = poolps.tile([gmax, 128], dt.float32)
                _edge_phase(tc, ins, meta, layer=2, table=table2, ald=ald2,
                            out_slot=None, wpool=wpool, pool_psum=pool_ps)

                # ---------------- P8: head ----------------
                pooled = poolp.tile([gmax, 128], dt.float32)
                nc.vector.tensor_copy(out=pooled[:], in_=pool_ps[:])
                b2g = poolp.tile([gmax, 128], dt.float32)
                nc.sync.dma_start(out=b2g[:], in_=ins["b2g"][:])
                nc.vector.tensor_tensor(out=pooled[:], in0=pooled[:], in1=b2g[:], op=OP.add)
                with tc.tile_pool(name="hps", bufs=1, space="PSUM") as hps:
                    pT_ps = hps.tile([P, gmax], dt.float32)
                    nc.tensor.transpose(out=pT_ps[:], in_=pooled[:],
                                        identity=ident[:gmax, :gmax])
                    pT = poolp.tile([P, gmax], dt.float32)
                    nc.vector.tensor_copy(out=pT[:], in_=pT_ps[:])
                    fcw = poolp.tile([P, NCLS], dt.float32)
                    nc.sync.dma_start(out=fcw[:], in_=ins["fcw"][:])
                    lg_ps = hps.tile([gmax, NCLS], dt.float32)
                    nc.tensor.matmul(out=lg_ps[:], lhsT=pT[:], rhs=fcw[:],
                                     start=True, stop=True)
                    lg = poolp.tile([gmax, NCLS], dt.float32)
                    nc.vector.tensor_copy(out=lg[:], in_=lg_ps[:])
                fcb = poolp.tile([gmax, NCLS], dt.float32)
                nc.sync.dma_start(out=fcb[:], in_=ins["fcb_bc"][:])
                nc.vector.tensor_tensor(out=lg[:], in0=lg[:], in1=fcb[:], op=OP.add)
                # log_softmax
                m = poolp.tile([gmax, 1], dt.float32)
                nc.vector.tensor_reduce(out=m[:], in_=lg[:], axis=AX.X, op=OP.max)
                nc.vector.tensor_scalar(out=lg[:], in0=lg[:], scalar1=m[:],
                                        scalar2=None, op0=OP.subtract)
                ex = poolp.tile([gmax, NCLS], dt.float32)
                nc.scalar.activation(out=ex[:], in_=lg[:], func=ACTF.Exp)
                ss = poolp.tile([gmax, 1], dt.float32)
                nc.vector.tensor_reduce(out=ss[:], in_=ex[:], axis=AX.X, op=OP.add)
                nc.scalar.activation(out=ss[:], in_=ss[:], func=ACTF.Ln)
                nc.vector.tensor_scalar(out=lg[:], in0=lg[:], scalar1=ss[:],
                                        scalar2=None, op0=OP.subtract)
                nc.sync.dma_start(out=ins["out"][:], in_=lg[:])


def _edge_phase(tc, ins, meta, layer, table, ald, out_slot, wpool, pool_psum):
    import concourse.bass as bass
    import concourse.mybir as mybir

    nc = tc.nc
    dt = mybir.dt
    AX = mybir.AxisListType
    OP = mybir.AluOpType
    ACTF = mybir.ActivationFunctionType
    gmax = meta["gmax"]
    NCOL = meta["NCOL"]

    if layer == 1:
        R, F, H, SBMAX, NBMAX = 80, 64, 8, 128, 32
        idx_in, HOFF = ins["idx1"], 64
        batches = meta["batches1"]
    else:
        R, F, H, SBMAX, NBMAX = 130, 128, 1, 96, 24
        idx_in, HOFF = ins["idx2"], 128
        batches = meta["batches2"]

    first_pool = [True]

    with tc.tile_pool(name=f"ed{layer}", bufs=3 if layer == 1 else 2) as ep, \
         tc.tile_pool(name=f"eds{layer}", bufs=4) as eps:
        for (w, j0, ncols, ec0) in batches:
            Sb = ncols * w
            idx = eps.tile([P, Sb], dt.int32, tag="idx")
            nc.sync.dma_start(out=idx[:], in_=idx_in[:, ec0:ec0 + Sb])
            mn = eps.tile([P, Sb], dt.float32, tag="mn")
            nc.sync.dma_start(out=mn[:], in_=ins["maskneg"][:, ec0:ec0 + Sb])
            ed = ep.tile([P, Sb * R], dt.float32, tag="ed")
            for s in range(Sb):
                nc.gpsimd.indirect_dma_start(
                    out=ed[:, s * R:(s + 1) * R], out_offset=None, in_=table[:],
                    in_offset=bass.IndirectOffsetOnAxis(ap=idx[:, s:s + 1], axis=0))

            edv = ed[:].rearrange("p (n k r) -> p n k r", k=w, r=R)
            et = eps.tile([P, Sb * H], dt.float32, tag="et")
            etv = et[:].rearrange("p (n k h) -> p n k h", k=w, h=H)
            # e = al_s[src] + al_d[dst]
            aldv = (ald[:].rearrange("p (n o h) -> p n o h", o=1, h=H)
                    [:, j0:j0 + ncols].to_broadcast([P, ncols, w, H]))
            nc.vector.tensor_tensor(out=etv, in0=edv[:, :, :, HOFF:HOFF + H],
                                    in1=aldv, op=OP.add)
            # exp(leaky_relu(e)), then zero padding slots exactly
            # (ACT's Lrelu ignores alpha on HW: leaky = max(x, 0.2x) on DVE)
            lt = eps.tile([P, Sb * H], dt.float32, tag="lt")
            nc.vector.tensor_scalar(out=lt[:], in0=et[:], scalar1=0.2,
                                    scalar2=None, op0=OP.mult)
            nc.vector.tensor_tensor(out=et[:], in0=et[:], in1=lt[:], op=OP.max)
            nc.scalar.activation(out=et[:], in_=et[:], func=ACTF.Exp)
            mnv = (mn[:].rearrange("p (n k o) -> p n k o", k=w, o=1)
                   .to_broadcast([P, ncols, w, H]))
            nc.vector.tensor_tensor(out=etv, in0=etv, in1=mnv, op=OP.mult)
            # s[d] = sum_k exp
            s = eps.tile([P, ncols * H], dt.float32, tag="s")
            sv = s[:]
            nc.vector.tensor_reduce(
                out=sv.rearrange("p (n h) -> p n h", h=H),
                in_=et[:].rearrange("p (n k h) -> p n h k", k=w, h=H),
                axis=AX.X, op=OP.add)
            nc.vector.tensor_scalar(out=sv, in0=sv, scalar1=1e-16, scalar2=None,
                                    op0=OP.add)
            nc.vector.reciprocal(out=sv, in_=sv)
            # WH = h[src] * exp  (in place on the h part of ed)
            hview = ed[:].rearrange("p (n r) -> p n r", r=R)[:, :, 0:F]
            if H == 1:
                exv = (et[:].rearrange("p (n o) -> p n o", o=1)
                       .to_broadcast([P, Sb, F]))
            else:
                exv = (et[:].rearrange("p (n h o) -> p n h o", h=H, o=1)
                       .to_broadcast([P, Sb, H, F // H]))
                hview = (ed[:].rearrange("p (n r) -> p n r", r=R)
                         [:, :, 0:F].rearrange("p n (h c) -> p n h c", h=H))
            nc.vector.tensor_tensor(out=hview, in0=hview, in1=exv, op=OP.mult)
            # out[d] = sum_k WH / s[d]
            if layer == 1:
                ov = (out_slot[:].rearrange("p (n f) -> p n f", f=F)
                      [:, j0:j0 + ncols])
                x2b = None
            else:
                x2b = ep.tile([P, ncols * F], dt.float32, tag="x2b")
                ov = x2b[:].rearrange("p (n f) -> p n f", f=F)
            nc.vector.tensor_reduce(
                out=ov,
                in_=ed[:].rearrange("p (n k r) -> p n r k", k=w, r=R)[:, :, 0:F, :],
                axis=AX.X, op=OP.add)
            if H == 1:
                sinvv = (s[:].rearrange("p (n o) -> p n o", o=1)
                         .to_broadcast([P, ncols, F]))
                ovv = ov
            else:
                sinvv = (s[:].rearrange("p (n h o) -> p n h o", h=H, o=1)
                         .to_broadcast([P, ncols, H, F // H]))
                ovv = ov.rearrange("p n (h c) -> p n h c", h=H)
            nc.vector.tensor_tensor(out=ovv, in0=ovv, in1=sinvv, op=OP.mult)

            if layer == 2:
                for jj in range(ncols):
                    nc.tensor.matmul(
                        out=pool_psum[:],
                        lhsT=wpool[:, (j0 + jj) * gmax:(j0 + jj + 1) * gmax],
                        rhs=x2b[:, jj * F:(jj + 1) * F],
                        start=first_pool[0],
                        stop=(j0 + jj == NCOL - 1) or
                             ((w, j0, ncols, ec0) == batches[-1] and jj == ncols - 1),
                        skip_group_check=True)
                    first_pool[0] = False


# ----------------------------------------------------------------------------
# runner
# ----------------------------------------------------------------------------

_CACHE = {}


def _get_nc(meta, shapes):
    key = str(sorted(meta.items(), key=lambda kv: kv[0]))
    if key in _CACHE:
        return _CACHE[key]
    import concourse.bacc as bacc
    import concourse.tile as tile
    import concourse.mybir as mybir
    dt = mybir.dt
    nc = bacc.Bacc("TRN2", target_bir_lowering=False, debug=False,
                   num_devices=NCORES)
    dts = {"idx1": dt.int32, "idx2": dt.int32, "ownrow1": dt.int32,
           "ownrow2": dt.int32}
    ins = {}
    for name, shape in shapes.items():
        ins[name] = nc.dram_tensor(name, list(shape),
                                   dts.get(name, dt.float32),
                                   kind="ExternalInput").ap()
    ins["out"] = nc.dram_tensor("out", [meta["gmax"], NCLS], dt.float32,
                                kind="ExternalOutput").ap()
    if os.environ.get("GAT_DEBUG"):
        ins["dbg_x1"] = nc.dram_tensor("dbg_x1", [P, meta["NCOL"] * 64],
                                       dt.float32, kind="ExternalOutput").ap()
        ins["dbg_ald1"] = nc.dram_tensor("dbg_ald1", [P, meta["NCOL"] * 8],
                                         dt.float32, kind="ExternalOutput").ap()
    with tile.TileContext(nc) as tc:
        build_program(tc, ins, meta)
    nc.compile()
    _CACHE[key] = nc
    return nc


def make_inputs(x, edge_index, batch, W1, a_src1, a_dst1, b1, W2, a_src2,
                a_dst2, b2, fc_w, fc_b):
    x = np.asarray(x, np.float32)
    host, meta, aux = _prep(x, np.asarray(edge_index), np.asarray(batch))
    NPAD1 = meta["NPAD1"]
    xT = np.zeros((P, NPAD1), np.float32)
    xT[:, :meta["N"]] = np.ascontiguousarray(x.T)
    shared = dict(
        xT=xT,
        W1=np.asarray(W1, np.float32),
        a1s_bc=np.tile(np.asarray(a_src1, np.float32).reshape(1, 64), (P, 1)),
        a1d_bc=np.tile(np.asarray(a_dst1, np.float32).reshape(1, 64), (P, 1)),
        b1bc=np.tile(np.asarray(b1, np.float32).reshape(1, 64), (P, 1)),
        W2=np.asarray(W2, np.float32),
        a2s_bc=np.tile(np.asarray(a_src2, np.float32).reshape(1, 128), (64, 1)),
        a2d_bc=np.tile(np.asarray(a_dst2, np.float32).reshape(1, 128), (64, 1)),
        fcw=np.asarray(fc_w, np.float32),
        fcb_bc=np.tile(np.asarray(fc_b, np.float32).reshape(1, NCLS),
                       (meta["gmax"], 1)),
    )
    in_maps = []
    for c in range(NCORES):
        m = dict(shared)
        m.update(host[c])
        # b2 contribution per graph (zero for empty graphs)
        g0, g1 = meta["g0"][c], meta["g1"][c]
        nonempty = np.zeros((meta["gmax"], 1), np.float32)
        ge = np.searchsorted(np.asarray(batch), np.arange(NG), side="left")
        gEnd = np.searchsorted(np.asarray(batch), np.arange(NG), side="right")
        cnt = (gEnd - ge)[g0:g1]
        nonempty[:g1 - g0, 0] = (cnt > 0).astype(np.float32)
        m["b2g"] = nonempty * np.asarray(b2, np.float32).reshape(1, 128)
        in_maps.append(m)
    return in_maps, meta, aux


def kernel(x, edge_index, batch, W1, a_src1, a_dst1, b1, W2, a_src2, a_dst2,
           b2, fc_w, fc_b):
    in_maps, meta, aux = make_inputs(x, edge_index, batch, W1, a_src1, a_dst1,
                                     b1, W2, a_src2, a_dst2, b2, fc_w, fc_b)
    global _LAST
    _LAST = dict(meta=meta, aux=aux)
    shapes = {k: v.shape for k, v in in_maps[0].items()}
    nc = _get_nc(meta, shapes)
    from concourse.bass_utils import run_bass_kernel_spmd
    res = run_bass_kernel_spmd(nc, in_maps, core_ids=list(range(NCORES)))
    _LAST["res"] = res
    out = np.zeros((NG, NCLS), np.float32)
    for c in range(NCORES):
        g0, g1 = meta["g0"][c], meta["g1"][c]
        out[g0:g1] = res.results[c]["out"][:g1 - g0]
    return out



# revision 8
# speedup vs baseline: 1.4101x; 1.3333x over previous
"""GAT (2-layer graph attention network + mean-pool + classifier) on 8 Trainium2
NeuronCores via Bass/Tile.

v3 design — minimize host->device bytes (the wall-clock is upload-dominated)
and device work:
- Unified slot-grid row space: node (core c, partition p, column j) lives at
  table row c*16384 + p*128 + j for BOTH layer tables. L1 and L2 use identical
  batch geometry, so ONE idx array serves both layers.
- x is uploaded pre-permuted into slot order, bf16, sharded per core
  (4.2MB/core instead of 51MB replicated f32).
- Each core projects only its own node shard (128 matmuls), then the shard
  tables are AllGathered on-device (bf16) to replicate for local gathers.
- Tables are bf16: halves gather bytes; softmax/accumulation stay f32.
- Edge aggregation: degree-class slot layout, k-OUTER ordering -> segment
  reductions are dense pairwise plane adds (bf16+bf16->f32), no strided
  reduces, no masks (pad slots point at a row whose attention logit is -300).
- Pooling weights built on device from compact gid/wval inputs.
"""

import os
import sys
import numpy as np

sys.path.insert(0, "/opt/trn_rl_repo")

P = 128          # partitions
NG = 256         # graphs
NCLS = 10        # classes
NCORES = 8
NCOL = 128       # node columns per core
SHROWS = P * NCOL  # table rows per core shard (16384)

CLASSES = [1, 2, 3, 4, 5, 6, 7, 8, 10, 12, 14, 16, 18, 20, 22, 24, 26, 28,
           30, 32, 36, 40, 44, 48, 56, 64, 80, 96]

SB, NB = 128, 32   # batch: max slots, max node-columns (both layers)
PADROW = 127       # slot (p=0, j=127) of core 0; column 127 is kept invalid


# ----------------------------------------------------------------------------
# host-side preprocessing (numpy only; index/layout work, no model math)
# ----------------------------------------------------------------------------

def _mk_batches(active, G_w, col0):
    """Batches of (w, j0, ncols, ec0); k-outer inside each batch:
    column ec0 + k*ncols + (j - j0)."""
    out = []
    ecur = 0
    class_e0 = {}
    class_step = {}
    for i in active:
        w = CLASSES[i]
        step = max(1, min(NB, SB // w))
        class_e0[i] = ecur
        class_step[i] = step
        j = 0
        while j < int(G_w[i]):
            nc_ = min(step, int(G_w[i]) - j)
            out.append((w, col0[i] + j, nc_, ecur))
            ecur += w * nc_
            j += nc_
    return out, ecur, class_e0, class_step


def _prep(x, edge_index, batch):
    import ml_dtypes
    N = x.shape[0]

    src = np.concatenate([edge_index[0], np.arange(N, dtype=np.int64)])
    dst = np.concatenate([edge_index[1], np.arange(N, dtype=np.int64)])
    batch = np.asarray(batch)

    gstart = np.searchsorted(batch, np.arange(NG), side="left")
    gend = np.searchsorted(batch, np.arange(NG), side="right")
    cum = gend.astype(np.float64)
    bounds = [0]
    for c in range(1, NCORES):
        bounds.append(int(np.searchsorted(cum, c * N / NCORES)))
    bounds.append(NG)
    g0 = np.array(bounds[:-1])
    g1 = np.array(bounds[1:])
    n0 = np.where(g0 < NG, gstart[np.minimum(g0, NG - 1)], N)
    n1 = np.where(g1 > 0, gend[np.minimum(g1 - 1, NG - 1)], 0)
    n0[0] = 0
    n1[-1] = N
    gmax = int((g1 - g0).max())

    order = np.argsort(dst, kind="stable")
    src_s, dst_s = src[order], dst[order]
    core_edges = []
    for c in range(NCORES):
        lo = np.searchsorted(dst_s, n0[c])
        hi = np.searchsorted(dst_s, n1[c])
        core_edges.append((src_s[lo:hi], dst_s[lo:hi] - n0[c]))

    cls_arr = np.array(CLASSES)
    counts = np.zeros((NCORES, len(CLASSES)), np.int64)
    degs = []
    for c in range(NCORES):
        nloc = int(n1[c] - n0[c])
        d = np.bincount(core_edges[c][1], minlength=nloc)
        assert d.min() >= 1 and d.max() <= CLASSES[-1], (d.min(), d.max())
        degs.append(d)
        ci = np.searchsorted(cls_arr, d)
        counts[c] = np.bincount(ci, minlength=len(CLASSES))
    G_w = np.maximum.reduce([(counts[c] + P - 1) // P for c in range(NCORES)])
    active = [i for i in range(len(CLASSES)) if counts[:, i].max() > 0]
    col0 = {}
    ncol_total = 0
    for i in active:
        col0[i] = ncol_total
        ncol_total += int(G_w[i])
    assert ncol_total <= NCOL - 1, ncol_total   # column 127 stays invalid

    batches, S, ce0, step = _mk_batches(active, G_w, col0)

    # slot assignment (per core), global node -> (core, p, j)
    g_core = np.zeros(N, np.int32)
    g_p = np.zeros(N, np.int32)
    g_j = np.zeros(N, np.int32)
    per_core = []
    for c in range(NCORES):
        d = degs[c]
        ci = np.searchsorted(cls_arr, d)
        esrc, edst = core_edges[c]
        eorder = np.lexsort((esrc, edst))
        esrc = esrc[eorder]
        edst = edst[eorder]

        slot_node = np.full((P, NCOL), -1, np.int64)
        e_p = np.zeros(len(esrc), np.int64)
        e_col = np.zeros(len(esrc), np.int64)
        for i in active:
            w = CLASSES[i]
            nodes = np.nonzero(ci == i)[0]
            if len(nodes) == 0:
                continue
            s = np.arange(len(nodes))
            pp = s % P
            jrel = s // P
            slot_node[pp, col0[i] + jrel] = nodes
            g_core[n0[c] + nodes] = c
            g_p[n0[c] + nodes] = pp
            g_j[n0[c] + nodes] = col0[i] + jrel
            emask = ci[edst] == i
            eidx = np.nonzero(emask)[0]
            dn = d[nodes]
            t = np.repeat(s, dn)
            starts = np.concatenate([[0], np.cumsum(dn)[:-1]])
            k = np.arange(len(eidx)) - np.repeat(starts, dn)
            jr = jrel[t]
            q = jr // step[i]
            ncols_q = np.minimum(step[i], int(G_w[i]) - q * step[i])
            e_p[eidx] = pp[t]
            e_col[eidx] = (ce0[i] + w * q * step[i] + k * ncols_q
                           + (jr - q * step[i]))
        per_core.append(dict(slot_node=slot_node, esrc=esrc, e_p=e_p,
                             e_col=e_col))

    # second pass: idx (needs global slot map), per-core uploads
    host = []
    cnt = (gend - gstart).astype(np.float32)
    for c in range(NCORES):
        pc = per_core[c]
        sl = pc["esrc"]
        row = (g_core[sl].astype(np.int64) * SHROWS
               + g_p[sl].astype(np.int64) * NCOL + g_j[sl])
        idx = np.full((P, S), PADROW, np.int32)
        idx[pc["e_p"], pc["e_col"]] = row.astype(np.int32)

        sn = pc["slot_node"]
        valid = sn >= 0
        nidx = np.where(valid, sn, 0)

        # x permuted into slot order, bf16, transposed to [feat, slot]
        xs = np.zeros((SHROWS, x.shape[1]), np.float32)
        ppi, jji = np.nonzero(valid)
        xs[jji * P + ppi] = x[n0[c] + sn[ppi, jji]]
        xT = np.ascontiguousarray(xs.T).astype(ml_dtypes.bfloat16)

        ownbase = (np.arange(P, dtype=np.int32) * NCOL
                   + c * SHROWS).reshape(P, 1)

        gnode = batch[np.minimum(nidx + n0[c], N - 1)]
        gl = (gnode - g0[c]).astype(np.int64)
        ok = valid & (gl >= 0) & (gl < gmax)
        gid = np.where(ok, gl, -1).astype(np.float32)
        wval = np.where(ok, 1.0 / np.maximum(cnt[np.minimum(gnode, NG - 1)],
                                             1.0), 0.0).astype(np.float32)

        host.append(dict(idx=idx, xT=xT, ownbase=ownbase,
                         gid=gid, wval=wval))

    meta = dict(
        N=N, S=S, gmax=gmax, batches=batches,
        n0=n0.tolist(), n1=n1.tolist(), g0=g0.tolist(), g1=g1.tolist(),
    )
    aux = dict(slot_nodes=[pc["slot_node"] for pc in per_core],
               g_core=g_core, g_p=g_p, g_j=g_j)
    return host, meta, aux


# ----------------------------------------------------------------------------
# program builder
# ----------------------------------------------------------------------------

def build_program(tc, ins, meta):
    import concourse.bass as bass
    import concourse.mybir as mybir
    from concourse.masks import make_identity

    nc = tc.nc
    dt = mybir.dt
    AX = mybir.AxisListType
    OP = mybir.AluOpType
    ACTF = mybir.ActivationFunctionType

    gmax = meta["gmax"]
    R1, R2 = 80, 130           # table row widths (h + al_s + al_d)

    # DRAM scratch (plain Internal; indirect DMA needs offset-0 tensors)
    t1shard = nc.dram_tensor("t1shard", [SHROWS, R1], dt.bfloat16, kind="Internal").ap()
    table1 = nc.dram_tensor("table1", [SHROWS * NCORES, R1], dt.bfloat16, kind="Internal").ap()
    t2shard = nc.dram_tensor("t2shard", [SHROWS, R2], dt.bfloat16, kind="Internal").ap()
    table2 = nc.dram_tensor("table2", [SHROWS * NCORES, R2], dt.bfloat16, kind="Internal").ap()

    with tc.tile_pool(name="cst", bufs=1) as cst:
        # ---------------- constants / fused weights ----------------
        rhs1f = cst.tile([P, 80], dt.float32)
        w1 = cst.tile([P, 64], dt.float32)
        nc.sync.dma_start(out=w1[:], in_=ins["W1"][:])
        nc.vector.tensor_copy(out=rhs1f[:, 0:64], in_=w1[:])
        a1s = cst.tile([P, 64], dt.float32)
        a1d = cst.tile([P, 64], dt.float32)
        nc.sync.dma_start(out=a1s[:], in_=ins["a1s_bc"][:])
        nc.sync.dma_start(out=a1d[:], in_=ins["a1d_bc"][:])
        tmp1 = cst.tile([P, 64], dt.float32)
        nc.vector.tensor_tensor(out=tmp1[:], in0=w1[:], in1=a1s[:], op=OP.mult)
        nc.vector.tensor_reduce(
            out=rhs1f[:, 64:72], in_=tmp1[:].rearrange("p (h c) -> p h c", c=8),
            axis=AX.X, op=OP.add)
        nc.vector.tensor_tensor(out=tmp1[:], in0=w1[:], in1=a1d[:], op=OP.mult)
        nc.vector.tensor_reduce(
            out=rhs1f[:, 72:80], in_=tmp1[:].rearrange("p (h c) -> p h c", c=8),
            axis=AX.X, op=OP.add)
        rhs1 = cst.tile([P, 80], dt.bfloat16)
        nc.vector.tensor_copy(out=rhs1[:], in_=rhs1f[:])

        rhs2f = cst.tile([64, R2], dt.float32)
        w2 = cst.tile([64, 128], dt.float32)
        nc.sync.dma_start(out=w2[:], in_=ins["W2"][:])
        nc.vector.tensor_copy(out=rhs2f[:, 0:128], in_=w2[:])
        a2s = cst.tile([64, 128], dt.float32)
        a2d = cst.tile([64, 128], dt.float32)
        nc.sync.dma_start(out=a2s[:], in_=ins["a2s_bc"][:])
        nc.sync.dma_start(out=a2d[:], in_=ins["a2d_bc"][:])
        tmp2 = cst.tile([64, 128], dt.float32)
        nc.vector.tensor_tensor(out=tmp2[:], in0=w2[:], in1=a2s[:], op=OP.mult)
        nc.vector.tensor_reduce(out=rhs2f[:, 128:129], in_=tmp2[:], axis=AX.X, op=OP.add)
        nc.vector.tensor_tensor(out=tmp2[:], in0=w2[:], in1=a2d[:], op=OP.mult)
        nc.vector.tensor_reduce(out=rhs2f[:, 129:130], in_=tmp2[:], axis=AX.X, op=OP.add)
        rhs2 = cst.tile([64, R2], dt.bfloat16)
        nc.vector.tensor_copy(out=rhs2[:], in_=rhs2f[:])

        b1bc = cst.tile([P, 64], dt.float32)
        nc.sync.dma_start(out=b1bc[:], in_=ins["b1bc"][:])
        ident = cst.tile([P, P], dt.float32)
        make_identity(nc, ident[:])

        # pad-row constant: attention logit -300 (exp -> 0); -300 is exact bf16
        padc = cst.tile([1, 8], dt.bfloat16)
        nc.scalar.memzero(padc[:])
        nc.vector.tensor_scalar(out=padc[:], in0=padc[:], scalar1=-300.0,
                                scalar2=None, op0=OP.add)

        own = cst.tile([P, 1], dt.int32)
        nc.sync.dma_start(out=own[:], in_=ins["ownbase"][:])
        ald1 = cst.tile([P, NCOL * 8], dt.float32)
        ald2 = cst.tile([P, NCOL], dt.float32)

        with tc.tile_pool(name="slotp", bufs=1) as slotp:
            x1slot = slotp.tile([P, NCOL * 64], dt.float32)
            nc.scalar.memzero(x1slot[:])

            # ---------------- P1: own-shard L1 projection ----------------
            with tc.tile_pool(name="p1", bufs=3) as p1, \
                 tc.tile_pool(name="p1ps", bufs=4, space="PSUM") as p1ps:
                GT = 6
                t = 0
                while t < NCOL:
                    g = min(GT, NCOL - t)
                    xt = p1.tile([P, GT * P], dt.bfloat16, tag="xt")
                    nc.sync.dma_start(out=xt[:, :g * P],
                                      in_=ins["xT"][:, t * P:(t + g) * P])
                    ps = p1ps.tile([P, GT * R1], dt.float32, tag="ps")
                    for i in range(g):
                        nc.tensor.matmul(out=ps[:, i * R1:(i + 1) * R1],
                                         lhsT=xt[:, i * P:(i + 1) * P],
                                         rhs=rhs1[:], start=True, stop=True)
                    st = p1.tile([P, GT * R1], dt.bfloat16, tag="st")
                    nc.vector.tensor_copy(out=st[:, :g * R1], in_=ps[:, :g * R1])
                    nc.sync.dma_start(
                        out=t1shard[:].rearrange("(p t) r -> p (t r)", p=P)[:, t * R1:(t + g) * R1],
                        in_=st[:, :g * R1])
                    t += g
            # pad row (local row 127 = slot (p=0, j=127), invalid by assert)
            nc.sync.dma_start(out=t1shard[PADROW:PADROW + 1, 64:72], in_=padc[0:1, :])

            # ---------------- AllGather table1 ----------------
            nc.gpsimd.collective_compute(
                "AllGather", mybir.AluOpType.bypass,
                replica_groups=[list(range(NCORES))],
                ins=[t1shard[:].opt()], outs=[table1[:].opt()])

            # ---------------- P2: al_d1 for own nodes (consecutive rows) ----
            with tc.tile_pool(name="p2", bufs=1) as p2:
                tmp = p2.tile([P, NCOL * R1], dt.bfloat16)
                nc.gpsimd.indirect_dma_start(
                    out=tmp[:], out_offset=None, in_=table1[:],
                    in_offset=bass.IndirectOffsetOnAxis(ap=own[:, 0:1], axis=0))
                nc.vector.tensor_copy(
                    out=ald1[:],
                    in_=tmp[:].rearrange("p (n r) -> p n r", r=R1)[:, :, 72:80])

            # ---------------- P3: L1 edge phase ----------------
            if not os.environ.get("GAT_NOEDGE"):
                _edge_phase(tc, ins, meta, layer=1, table=table1, ald=ald1,
                            out_slot=x1slot, wpool=None, pool_psum=None)

            # bias + relu
            nc.vector.tensor_tensor(
                out=x1slot[:].rearrange("p (n f) -> p n f", f=64),
                in0=x1slot[:].rearrange("p (n f) -> p n f", f=64),
                in1=b1bc[:].rearrange("p (o f) -> p o f", o=1).to_broadcast([P, NCOL, 64]),
                op=OP.add)
            nc.scalar.activation(out=x1slot[:], in_=x1slot[:], func=ACTF.Relu)

            if os.environ.get("GAT_STOP"):
                nc.sync.dma_start(out=ins["out"][:, 0:1],
                                  in_=x1slot[0:gmax, 0:1])
                return

            # ---------------- P4+P5: transpose, own-shard L2 projection ----
            with tc.tile_pool(name="p4", bufs=1) as p4:
                x1T = p4.tile([64, SHROWS], dt.bfloat16)
                with tc.tile_pool(name="p4ps", bufs=4, space="PSUM") as p4ps:
                    for j2 in range(0, NCOL, 2):
                        ps = p4ps.tile([64, 2 * P], dt.float32, tag="tp")
                        for k in range(2):
                            j = j2 + k
                            nc.tensor.transpose(
                                out=ps[:, k * P:(k + 1) * P],
                                in_=x1slot[:, j * 64:(j + 1) * 64], identity=ident[:])
                        nc.vector.tensor_copy(out=x1T[:, j2 * P:(j2 + 2) * P], in_=ps[:])
                with tc.tile_pool(name="p5", bufs=3) as p5, \
                     tc.tile_pool(name="p5ps", bufs=4, space="PSUM") as p5ps:
                    GP = 3
                    jp = 0
                    while jp < NCOL:
                        gp = min(GP, NCOL - jp)
                        ps = p5ps.tile([P, GP * R2], dt.float32, tag="ps2")
                        for i in range(gp):
                            nc.tensor.matmul(
                                out=ps[:, i * R2:(i + 1) * R2],
                                lhsT=x1T[:, (jp + i) * P:(jp + i + 1) * P],
                                rhs=rhs2[:], start=True, stop=True)
                        st = p5.tile([P, GP * R2], dt.bfloat16, tag="st2")
                        nc.vector.tensor_copy(out=st[:, :gp * R2], in_=ps[:, :gp * R2])
                        nc.sync.dma_start(
                            out=t2shard[:].rearrange("(p t) r -> p (t r)", p=P)[:, jp * R2:(jp + gp) * R2],
                            in_=st[:, :gp * R2])
                        jp += gp
            nc.sync.dma_start(out=t2shard[PADROW:PADROW + 1, 128:129],
                              in_=padc[0:1, 0:1])

            # ---------------- AllGather table2 ----------------
            nc.gpsimd.collective_compute(
                "AllGather", mybir.AluOpType.bypass,
                replica_groups=[list(range(NCORES))],
                ins=[t2shard[:].opt()], outs=[table2[:].opt()])

        # ---------------- P6: al_d2 for own nodes ----------------
        with tc.tile_pool(name="p6", bufs=1) as p6:
            tmp = p6.tile([P, NCOL * R2], dt.bfloat16)
            nc.gpsimd.indirect_dma_start(
                out=tmp[:], out_offset=None, in_=table2[:],
                in_offset=bass.IndirectOffsetOnAxis(ap=own[:, 0:1], axis=0))
            nc.vector.tensor_copy(
                out=ald2[:],
                in_=tmp[:].rearrange("p (n r) -> p n r", r=R2)[:, :, 129:130])

        if os.environ.get("GAT_STOP2"):
            nc.sync.dma_start(out=ins["out"][:, 0:1], in_=ald2[0:gmax, 0:1])
            return

        # ---------------- P7: L2 edge phase + pooling ----------------
        with tc.tile_pool(name="pool", bufs=1) as poolp, \
             tc.tile_pool(name="poolps", bufs=1, space="PSUM") as poolps:
            # build wpool[p, j, g] = wval[p,j] * (gid[p,j] == g) on device
            gid = poolp.tile([P, NCOL], dt.float32)
            nc.sync.dma_start(out=gid[:], in_=ins["gid"][:])
            wv = poolp.tile([P, NCOL], dt.float32)
            nc.sync.dma_start(out=wv[:], in_=ins["wval"][:])
            iota = poolp.tile([P, gmax], dt.float32)
            nc.sync.dma_start(out=iota[:], in_=ins["iota"][:])
            wpool = poolp.tile([P, NCOL * gmax], dt.float32)
            wpv = wpool[:].rearrange("p (n g) -> p n g", g=gmax)
            nc.vector.tensor_tensor(
                out=wpv,
                in0=(gid[:].rearrange("p (n o) -> p n o", o=1)
                     .to_broadcast([P, NCOL, gmax])),
                in1=(iota[:].rearrange("p (o g) -> p o g", o=1)
                     .to_broadcast([P, NCOL, gmax])),
                op=OP.is_equal)
            nc.vector.tensor_tensor(
                out=wpv, in0=wpv,
                in1=(wv[:].rearrange("p (n o) -> p n o", o=1)
                     .to_broadcast([P, NCOL, gmax])),
                op=OP.mult)

            pool_ps = poolps.tile([gmax, 128], dt.float32)
            _edge_phase(tc, ins, meta, layer=2, table=table2, ald=ald2,
                        out_slot=None, wpool=wpool, pool_psum=pool_ps)

            # ---------------- P8: head ----------------
            pooled = poolp.tile([gmax, 128], dt.float32)
            nc.vector.tensor_copy(out=pooled[:], in_=pool_ps[:])
            b2g = poolp.tile([gmax, 128], dt.float32)
            nc.sync.dma_start(out=b2g[:], in_=ins["b2g"][:])
            nc.vector.tensor_tensor(out=pooled[:], in0=pooled[:], in1=b2g[:], op=OP.add)
            with tc.tile_pool(name="hps", bufs=1, space="PSUM") as hps:
                pT_ps = hps.tile([P, gmax], dt.float32)
                nc.tensor.transpose(out=pT_ps[:], in_=pooled[:],
                                    identity=ident[:gmax, :gmax])
                pT = poolp.tile([P, gmax], dt.float32)
                nc.vector.tensor_copy(out=pT[:], in_=pT_ps[:])
                fcw = poolp.tile([P, NCLS], dt.float32)
                nc.sync.dma_start(out=fcw[:], in_=ins["fcw"][:])
                lg_ps = hps.tile([gmax, NCLS], dt.float32)
                nc.tensor.matmul(out=lg_ps[:], lhsT=pT[:], rhs=fcw[:],
                                 start=True, stop=True)
                lg = poolp.tile([gmax, NCLS], dt.float32)
                nc.vector.tensor_copy(out=lg[:], in_=lg_ps[:])
            fcb = poolp.tile([gmax, NCLS], dt.float32)
            nc.sync.dma_start(out=fcb[:], in_=ins["fcb_bc"][:])
            nc.vector.tensor_tensor(out=lg[:], in0=lg[:], in1=fcb[:], op=OP.add)
            # log_softmax
            m = poolp.tile([gmax, 1], dt.float32)
            nc.vector.tensor_reduce(out=m[:], in_=lg[:], axis=AX.X, op=OP.max)
            nc.vector.tensor_scalar(out=lg[:], in0=lg[:], scalar1=m[:],
                                    scalar2=None, op0=OP.subtract)
            ex = poolp.tile([gmax, NCLS], dt.float32)
            nc.scalar.activation(out=ex[:], in_=lg[:], func=ACTF.Exp)
            ss = poolp.tile([gmax, 1], dt.float32)
            nc.vector.tensor_reduce(out=ss[:], in_=ex[:], axis=AX.X, op=OP.add)
            nc.scalar.activation(out=ss[:], in_=ss[:], func=ACTF.Ln)
            nc.vector.tensor_scalar(out=lg[:], in0=lg[:], scalar1=ss[:],
                                    scalar2=None, op0=OP.subtract)
            nc.sync.dma_start(out=ins["out"][:], in_=lg[:])


def _edge_phase(tc, ins, meta, layer, table, ald, out_slot, wpool, pool_psum):
    import concourse.bass as bass
    import concourse.mybir as mybir

    nc = tc.nc
    dt = mybir.dt
    OP = mybir.AluOpType
    ACTF = mybir.ActivationFunctionType
    gmax = meta["gmax"]
    batches = meta["batches"]

    if layer == 1:
        R, F, H = 80, 64, 8
        HOFF = 64
    else:
        R, F, H = 130, 128, 1
        HOFF = 128
    C = F // H
    idx_in = ins["idx"]

    last = batches[-1]
    first = batches[0]

    with tc.tile_pool(name=f"ed{layer}", bufs=3 if layer == 1 else 2) as ep, \
         tc.tile_pool(name=f"eds{layer}", bufs=3) as eps:
        for (w, j0, ncols, ec0) in batches:
            Sb = ncols * w
            idx = eps.tile([P, SB], dt.int32, tag="idx")
            nc.sync.dma_start(out=idx[:, :Sb], in_=idx_in[:, ec0:ec0 + Sb])
            ed = ep.tile([P, SB * R], dt.bfloat16, tag="ed")
            for s in range(Sb):
                nc.gpsimd.indirect_dma_start(
                    out=ed[:, s * R:(s + 1) * R], out_offset=None, in_=table[:],
                    in_offset=bass.IndirectOffsetOnAxis(ap=idx[:, s:s + 1], axis=0))

            edk = ed[:, :Sb * R].rearrange("p (k n r) -> p k n r", k=w, r=R)
            eds_v = ed[:, :Sb * R].rearrange("p (s r) -> p s r", r=R)
            # e = al_s[src] + al_d[dst] (al_d identical across the w planes)
            et = eps.tile([P, SB * H], dt.float32, tag="et")
            etv = et[:, :Sb * H]
            nc.vector.tensor_copy(
                out=etv.rearrange("p (s h) -> p s h", h=H),
                in_=eds_v[:, :, HOFF:HOFF + H])
            aldv = (ald[:].rearrange("p (n h) -> p n h", h=H)[:, j0:j0 + ncols]
                    .rearrange("p (o n) h -> p o n h", o=1)
                    .to_broadcast([P, w, ncols, H]))
            nc.vector.tensor_tensor(
                out=etv.rearrange("p (k n h) -> p k n h", k=w, h=H),
                in0=etv.rearrange("p (k n h) -> p k n h", k=w, h=H),
                in1=aldv, op=OP.add)
            # exp(leaky_relu(e))  (leaky = max(x, 0.2x))
            lt = eps.tile([P, SB * H], dt.float32, tag="lt")
            nc.vector.tensor_scalar(out=lt[:, :Sb * H], in0=etv, scalar1=0.2,
                                    scalar2=None, op0=OP.mult)
            nc.vector.tensor_tensor(out=etv, in0=etv, in1=lt[:, :Sb * H], op=OP.max)
            nc.scalar.activation(out=etv, in_=etv, func=ACTF.Exp)
            # s[d] = sum_k exp : dense plane adds (f32)
            NH = ncols * H
            s = eps.tile([P, NB * 8], dt.float32, tag="s")
            sv = s[:, :NH]
            if w == 1:
                nc.vector.tensor_scalar(out=sv, in0=et[:, :NH], scalar1=1e-16,
                                        scalar2=None, op0=OP.add)
            else:
                nc.vector.tensor_tensor(out=sv, in0=et[:, 0:NH],
                                        in1=et[:, NH:2 * NH], op=OP.add)
                for k in range(2, w):
                    nc.vector.tensor_tensor(out=sv, in0=sv,
                                            in1=et[:, k * NH:(k + 1) * NH],
                                            op=OP.add)
                nc.vector.tensor_scalar(out=sv, in0=sv, scalar1=1e-16,
                                        scalar2=None, op0=OP.add)
            nc.vector.reciprocal(out=sv, in_=sv)
            # alpha in bf16 for the h multiply
            etb = eps.tile([P, SB * H], dt.bfloat16, tag="etb")
            nc.vector.tensor_copy(out=etb[:, :Sb * H], in_=etv)
            # WH = h[src] * alpha (in place, bf16)
            if H == 1:
                hview = eds_v[:, :, 0:F]
                exv = (etb[:, :Sb].rearrange("p (s o) -> p s o", o=1)
                       .to_broadcast([P, Sb, F]))
            else:
                hview = eds_v[:, :, 0:F].rearrange("p s (h c) -> p s h c", h=H)
                exv = (etb[:, :Sb * H].rearrange("p (s h o) -> p s h o", h=H, o=1)
                       .to_broadcast([P, Sb, H, C]))
            nc.vector.tensor_tensor(out=hview, in0=hview, in1=exv, op=OP.mult)

            # out[d] = (sum_k WH) / s[d] : pairwise bf16+bf16->f32 plane adds
            if layer == 1:
                ov = (out_slot[:].rearrange("p (n f) -> p n f", f=F)
                      [:, j0:j0 + ncols])
                x2b = None
            else:
                x2b = ep.tile([P, NB * F], dt.float32, tag="x2b")
                ov = x2b[:, :ncols * F].rearrange("p (n f) -> p n f", f=F)

            def plane(k):
                return (edk[:, k:k + 1, :, 0:F]
                        .rearrange("p o n r -> p (o n) r"))
            acc = eps.tile([P, NB * F], dt.float32, tag="acc")
            av = acc[:, :ncols * F].rearrange("p (n f) -> p n f", f=F)
            if w == 1:
                nc.vector.tensor_copy(out=ov, in_=plane(0))
            else:
                nc.vector.tensor_tensor(out=ov, in0=plane(0), in1=plane(1),
                                        op=OP.add)
                k = 2
                while k + 1 < w:
                    nc.vector.tensor_tensor(out=av, in0=plane(k),
                                            in1=plane(k + 1), op=OP.add)
                    nc.vector.tensor_tensor(out=ov, in0=ov, in1=av, op=OP.add)
                    k += 2
                if k < w:
                    nc.vector.tensor_copy(out=av, in_=plane(k))
                    nc.vector.tensor_tensor(out=ov, in0=ov, in1=av, op=OP.add)
            if H == 1:
                sinvv = (sv.rearrange("p (n o) -> p n o", o=1)
                         .to_broadcast([P, ncols, F]))
                ovv = ov
            else:
                sinvv = (sv.rearrange("p (n h o) -> p n h o", h=H, o=1)
                         .to_broadcast([P, ncols, H, C]))
                ovv = ov.rearrange("p n (h c) -> p n h c", h=H)
            nc.vector.tensor_tensor(out=ovv, in0=ovv, in1=sinvv, op=OP.mult)

            if layer == 2:
                for jj in range(ncols):
                    nc.tensor.matmul(
                        out=pool_psum[:],
                        lhsT=wpool[:, (j0 + jj) * gmax:(j0 + jj + 1) * gmax],
                        rhs=x2b[:, jj * F:(jj + 1) * F],
                        start=((w, j0, ncols, ec0) == first and jj == 0),
                        stop=((w, j0, ncols, ec0) == last and jj == ncols - 1),
                        skip_group_check=True)


# ----------------------------------------------------------------------------
# runner
# ----------------------------------------------------------------------------

_CACHE = {}


def _get_nc(meta, in_map0):
    key = str(sorted(meta.items(), key=lambda kv: kv[0]))
    if key in _CACHE:
        return _CACHE[key]
    import concourse.bacc as bacc
    import concourse.tile as tile
    import concourse.mybir as mybir
    dt = mybir.dt
    nc = bacc.Bacc("TRN2", target_bir_lowering=False, debug=False,
                   num_devices=NCORES)
    ins = {}
    for name, arr in in_map0.items():
        ins[name] = nc.dram_tensor(name, list(arr.shape),
                                   _np_dtype_to_bir(arr),
                                   kind="ExternalInput").ap()
    ins["out"] = nc.dram_tensor("out", [meta["gmax"], NCLS], dt.float32,
                                kind="ExternalOutput").ap()
    with tile.TileContext(nc) as tc:
        build_program(tc, ins, meta)
    nc.compile()
    _CACHE[key] = nc
    return nc


def _np_dtype_to_bir(a):
    import concourse.mybir as mybir
    import ml_dtypes
    dt = mybir.dt
    if a.dtype == np.int32:
        return dt.int32
    if a.dtype == ml_dtypes.bfloat16:
        return dt.bfloat16
    return dt.float32


def make_inputs(x, edge_index, batch, W1, a_src1, a_dst1, b1, W2, a_src2,
                a_dst2, b2, fc_w, fc_b):
    x = np.asarray(x, np.float32)
    host, meta, aux = _prep(x, np.asarray(edge_index), np.asarray(batch))
    gmax = meta["gmax"]
    iota = np.tile(np.arange(gmax, dtype=np.float32).reshape(1, gmax), (P, 1))
    shared = dict(
        W1=np.asarray(W1, np.float32),
        a1s_bc=np.tile(np.asarray(a_src1, np.float32).reshape(1, 64), (P, 1)),
        a1d_bc=np.tile(np.asarray(a_dst1, np.float32).reshape(1, 64), (P, 1)),
        b1bc=np.tile(np.asarray(b1, np.float32).reshape(1, 64), (P, 1)),
        W2=np.asarray(W2, np.float32),
        a2s_bc=np.tile(np.asarray(a_src2, np.float32).reshape(1, 128), (64, 1)),
        a2d_bc=np.tile(np.asarray(a_dst2, np.float32).reshape(1, 128), (64, 1)),
        fcw=np.asarray(fc_w, np.float32),
        fcb_bc=np.tile(np.asarray(fc_b, np.float32).reshape(1, NCLS),
                       (gmax, 1)),
        iota=iota,
    )
    in_maps = []
    for c in range(NCORES):
        m = dict(shared)
        m.update(host[c])
        g0, g1 = meta["g0"][c], meta["g1"][c]
        nonempty = np.zeros((gmax, 1), np.float32)
        ge = np.searchsorted(np.asarray(batch), np.arange(NG), side="left")
        gEnd = np.searchsorted(np.asarray(batch), np.arange(NG), side="right")
        cnt = (gEnd - ge)[g0:g1]
        nonempty[:g1 - g0, 0] = (cnt > 0).astype(np.float32)
        m["b2g"] = nonempty * np.asarray(b2, np.float32).reshape(1, 128)
        in_maps.append(m)
    return in_maps, meta, aux


def kernel(x, edge_index, batch, W1, a_src1, a_dst1, b1, W2, a_src2, a_dst2,
           b2, fc_w, fc_b):
    in_maps, meta, aux = make_inputs(x, edge_index, batch, W1, a_src1, a_dst1,
                                     b1, W2, a_src2, a_dst2, b2, fc_w, fc_b)
    global _LAST
    _LAST = dict(meta=meta, aux=aux)
    nc = _get_nc(meta, in_maps[0])
    from concourse.bass_utils import run_bass_kernel_spmd
    res = run_bass_kernel_spmd(nc, in_maps, core_ids=list(range(NCORES)))
    _LAST["res"] = res
    out = np.zeros((NG, NCLS), np.float32)
    for c in range(NCORES):
        g0, g1 = meta["g0"][c], meta["g1"][c]
        out[g0:g1] = res.results[c]["out"][:g1 - g0]
    return out


# revision 11
# speedup vs baseline: 1.4497x; 1.0281x over previous
"""GAT (2-layer graph attention network + mean-pool + classifier) on 8 Trainium2
NeuronCores via Bass/Tile.

v4 design — the wall-clock is dominated by the per-call host->device upload
and D2D collective bytes, so both are minimized:
- Inputs are TWO tensors per core: xT (own-shard node features, slot-ordered,
  bf16, only used columns) and one packed f32 "aux" blob carrying weights,
  pooling metadata and the gather index table (int32 bit-cast into f32).
- Unified slot-grid row space: node (core c, partition p, column j) lives at
  table row c*16384 + p*128 + j for BOTH layer tables; L1/L2 share one batch
  geometry so one idx array serves both layers.
- Each core projects only its own shard for layer 1; shards are AllGathered
  (72 cols: h|al_s) into the bf16 gather table. al_d for own nodes comes from
  a small local side table via a direct strided DMA (no gather).
- For layer 2 the (smaller) x1 activations are AllGathered and every core
  projects the full table2 locally.
- Edge aggregation: degree-class slot layout, k-OUTER ordering -> segment
  reductions are dense pairwise plane adds (bf16+bf16->f32), no masks (pad
  slots point at a row whose attention logit is -300).
"""

import os
import sys
import numpy as np

sys.path.insert(0, "/opt/trn_rl_repo")

P = 128          # partitions
NG = 256         # graphs
NCLS = 10        # classes
NCORES = 8
NCOL = 128       # node columns per core
SHROWS = P * NCOL  # table rows per core shard (16384)

CLASSES = [1, 2, 3, 4, 5, 6, 7, 8, 10, 12, 14, 16, 18, 20, 22, 24, 26, 28,
           30, 32, 36, 40, 44, 48, 56, 64, 80, 96]

SB, NB = 128, 32   # batch: max slots, max node-columns (both layers)
PADROW = 127       # slot (p=0, j=127) of core 0; column 127 is kept invalid


def _aux_layout(meta):
    """Column offsets of the packed per-core aux tensor (f32 view)."""
    gmax, S = meta["gmax"], meta["S"]
    off = {}
    cur = 0
    for name, width in [("gid", NCOL), ("wval", NCOL), ("iota", gmax),
                        ("own", 1), ("b2g", 128), ("fcb", NCLS),
                        ("W1", 64), ("a1s", 64), ("a1d", 64), ("b1", 64),
                        ("W2", 128), ("a2s", 128), ("a2d", 128),
                        ("fcw", NCLS), ("idx", S)]:
        off[name] = cur
        cur += width
    off["_total"] = cur
    return off


# ----------------------------------------------------------------------------
# host-side preprocessing (numpy only; index/layout work, no model math)
# ----------------------------------------------------------------------------

def _mk_batches(active, G_w, col0):
    out = []
    ecur = 0
    class_e0 = {}
    class_step = {}
    for i in active:
        w = CLASSES[i]
        step = max(1, min(NB, SB // w))
        class_e0[i] = ecur
        class_step[i] = step
        j = 0
        while j < int(G_w[i]):
            nc_ = min(step, int(G_w[i]) - j)
            out.append((w, col0[i] + j, nc_, ecur))
            ecur += w * nc_
            j += nc_
    return out, ecur, class_e0, class_step


def _prep(x, edge_index, batch):
    N = x.shape[0]

    src = np.concatenate([edge_index[0], np.arange(N, dtype=np.int64)])
    dst = np.concatenate([edge_index[1], np.arange(N, dtype=np.int64)])
    batch = np.asarray(batch)

    gstart = np.searchsorted(batch, np.arange(NG), side="left")
    gend = np.searchsorted(batch, np.arange(NG), side="right")
    cum = gend.astype(np.float64)
    bounds = [0]
    for c in range(1, NCORES):
        bounds.append(int(np.searchsorted(cum, c * N / NCORES)))
    bounds.append(NG)
    g0 = np.array(bounds[:-1])
    g1 = np.array(bounds[1:])
    n0 = np.where(g0 < NG, gstart[np.minimum(g0, NG - 1)], N)
    n1 = np.where(g1 > 0, gend[np.minimum(g1 - 1, NG - 1)], 0)
    n0[0] = 0
    n1[-1] = N
    gmax = int((g1 - g0).max())

    order = np.argsort(dst, kind="stable")
    src_s, dst_s = src[order], dst[order]
    core_edges = []
    for c in range(NCORES):
        lo = np.searchsorted(dst_s, n0[c])
        hi = np.searchsorted(dst_s, n1[c])
        core_edges.append((src_s[lo:hi], dst_s[lo:hi] - n0[c]))

    cls_arr = np.array(CLASSES)
    counts = np.zeros((NCORES, len(CLASSES)), np.int64)
    degs = []
    for c in range(NCORES):
        nloc = int(n1[c] - n0[c])
        d = np.bincount(core_edges[c][1], minlength=nloc)
        assert d.min() >= 1 and d.max() <= CLASSES[-1], (d.min(), d.max())
        degs.append(d)
        ci = np.searchsorted(cls_arr, d)
        counts[c] = np.bincount(ci, minlength=len(CLASSES))
    G_w = np.maximum.reduce([(counts[c] + P - 1) // P for c in range(NCORES)])
    active = [i for i in range(len(CLASSES)) if counts[:, i].max() > 0]
    col0 = {}
    ncol_total = 0
    for i in active:
        col0[i] = ncol_total
        ncol_total += int(G_w[i])
    assert ncol_total <= NCOL - 1, ncol_total   # column 127 stays invalid

    batches, S, ce0, step = _mk_batches(active, G_w, col0)

    g_core = np.zeros(N, np.int32)
    g_p = np.zeros(N, np.int32)
    g_j = np.zeros(N, np.int32)
    per_core = []
    for c in range(NCORES):
        d = degs[c]
        ci = np.searchsorted(cls_arr, d)
        esrc, edst = core_edges[c]
        eorder = np.lexsort((esrc, edst))
        esrc = esrc[eorder]
        edst = edst[eorder]

        slot_node = np.full((P, NCOL), -1, np.int64)
        e_p = np.zeros(len(esrc), np.int64)
        e_col = np.zeros(len(esrc), np.int64)
        for i in active:
            w = CLASSES[i]
            nodes = np.nonzero(ci == i)[0]
            if len(nodes) == 0:
                continue
            s = np.arange(len(nodes))
            pp = s % P
            jrel = s // P
            slot_node[pp, col0[i] + jrel] = nodes
            g_core[n0[c] + nodes] = c
            g_p[n0[c] + nodes] = pp
            g_j[n0[c] + nodes] = col0[i] + jrel
            emask = ci[edst] == i
            eidx = np.nonzero(emask)[0]
            dn = d[nodes]
            t = np.repeat(s, dn)
            starts = np.concatenate([[0], np.cumsum(dn)[:-1]])
            k = np.arange(len(eidx)) - np.repeat(starts, dn)
            jr = jrel[t]
            q = jr // step[i]
            ncols_q = np.minimum(step[i], int(G_w[i]) - q * step[i])
            e_p[eidx] = pp[t]
            e_col[eidx] = (ce0[i] + w * q * step[i] + k * ncols_q
                           + (jr - q * step[i]))
        per_core.append(dict(slot_node=slot_node, esrc=esrc, e_p=e_p,
                             e_col=e_col))

    meta = dict(
        N=N, S=S, gmax=gmax, ncu=ncol_total, batches=batches,
        n0=n0.tolist(), n1=n1.tolist(), g0=g0.tolist(), g1=g1.tolist(),
    )

    host = []
    cnt = (gend - gstart).astype(np.float32)
    lay = _aux_layout(meta)
    for c in range(NCORES):
        pc = per_core[c]
        sl = pc["esrc"]
        row = (g_core[sl].astype(np.int64) * SHROWS
               + g_p[sl].astype(np.int64) * NCOL + g_j[sl])
        idx = np.full((P, S), PADROW, np.int32)
        idx[pc["e_p"], pc["e_col"]] = row.astype(np.int32)

        sn = pc["slot_node"]
        valid = sn >= 0
        nidx = np.where(valid, sn, 0)

        gnode = batch[np.minimum(nidx + n0[c], N - 1)]
        gl = (gnode - g0[c]).astype(np.int64)
        ok = valid & (gl >= 0) & (gl < gmax)
        gid = np.where(ok, gl, -1).astype(np.float32)
        wval = np.where(ok, 1.0 / np.maximum(cnt[np.minimum(gnode, NG - 1)],
                                             1.0), 0.0).astype(np.float32)
        ownbase = (np.arange(P, dtype=np.int32) * NCOL
                   + c * SHROWS).reshape(P, 1)
        host.append(dict(idx=idx, gid=gid, wval=wval, ownbase=ownbase,
                         valid=valid, nidx=nidx))

    aux_meta = dict(lay=lay)
    aux = dict(slot_nodes=[pc["slot_node"] for pc in per_core],
               g_core=g_core, g_p=g_p, g_j=g_j, host=host, aux_meta=aux_meta)
    return host, meta, aux


# ----------------------------------------------------------------------------
# program builder
# ----------------------------------------------------------------------------

def build_program(tc, ins, meta):
    import concourse.bass as bass
    import concourse.mybir as mybir
    from concourse.masks import make_identity

    nc = tc.nc
    dt = mybir.dt
    AX = mybir.AxisListType
    OP = mybir.AluOpType
    ACTF = mybir.ActivationFunctionType

    gmax = meta["gmax"]
    ncu = meta["ncu"]
    S = meta["S"]
    lay = _aux_layout(meta)
    R1, R1F, R2 = 72, 80, 130   # gather row widths; R1F = h|al_s|al_d

    t1shard = nc.dram_tensor("t1shard", [SHROWS, R1], dt.bfloat16, kind="Internal").ap()
    aldsh = nc.dram_tensor("aldsh", [SHROWS, 8], dt.bfloat16, kind="Internal").ap()
    table1 = nc.dram_tensor("table1", [SHROWS * NCORES, R1], dt.bfloat16, kind="Internal").ap()
    x1sh = nc.dram_tensor("x1sh", [64, SHROWS], dt.bfloat16, kind="Internal").ap()
    x1full = nc.dram_tensor("x1full", [64 * NCORES, SHROWS], dt.bfloat16, kind="Internal").ap()
    table2 = nc.dram_tensor("table2", [SHROWS * NCORES, R2], dt.bfloat16, kind="Internal").ap()

    if os.environ.get("GAT_NULL"):
        with tc.tile_pool(name="nullp", bufs=1) as np_:
            z = np_.tile([gmax, NCLS], dt.float32)
            nc.scalar.memzero(z[:])
            nc.sync.dma_start(out=ins["out"][:], in_=z[:])
        return

    with tc.tile_pool(name="cst", bufs=1) as cst:
        aux = cst.tile([P, lay["_total"]], dt.float32)
        nc.sync.dma_start(out=aux[:], in_=ins["aux"][:])

        def af(name, width, p0=0, pn=P):
            return aux[p0:pn, lay[name]:lay[name] + width]

        # ---------------- fused weights ----------------
        rhs1f = cst.tile([P, 80], dt.float32)
        nc.vector.tensor_copy(out=rhs1f[:, 0:64], in_=af("W1", 64))
        tmp1 = cst.tile([P, 64], dt.float32)
        nc.vector.tensor_tensor(out=tmp1[:], in0=af("W1", 64),
                                in1=af("a1s", 64), op=OP.mult)
        nc.vector.tensor_reduce(
            out=rhs1f[:, 64:72], in_=tmp1[:].rearrange("p (h c) -> p h c", c=8),
            axis=AX.X, op=OP.add)
        nc.vector.tensor_tensor(out=tmp1[:], in0=af("W1", 64),
                                in1=af("a1d", 64), op=OP.mult)
        nc.vector.tensor_reduce(
            out=rhs1f[:, 72:80], in_=tmp1[:].rearrange("p (h c) -> p h c", c=8),
            axis=AX.X, op=OP.add)
        rhs1 = cst.tile([P, 80], dt.bfloat16)
        nc.vector.tensor_copy(out=rhs1[:], in_=rhs1f[:])

        rhs2f = cst.tile([64, R2], dt.float32)
        nc.vector.tensor_copy(out=rhs2f[:, 0:128], in_=af("W2", 128, 0, 64))
        tmp2 = cst.tile([64, 128], dt.float32)
        nc.vector.tensor_tensor(out=tmp2[:], in0=af("W2", 128, 0, 64),
                                in1=af("a2s", 128, 0, 64), op=OP.mult)
        nc.vector.tensor_reduce(out=rhs2f[:, 128:129], in_=tmp2[:], axis=AX.X, op=OP.add)
        nc.vector.tensor_tensor(out=tmp2[:], in0=af("W2", 128, 0, 64),
                                in1=af("a2d", 128, 0, 64), op=OP.mult)
        nc.vector.tensor_reduce(out=rhs2f[:, 129:130], in_=tmp2[:], axis=AX.X, op=OP.add)
        rhs2 = cst.tile([64, R2], dt.bfloat16)
        nc.vector.tensor_copy(out=rhs2[:], in_=rhs2f[:])

        ident = cst.tile([P, P], dt.float32)
        make_identity(nc, ident[:])
        padc = cst.tile([1, 8], dt.bfloat16)
        nc.scalar.memzero(padc[:])
        nc.vector.tensor_scalar(out=padc[:], in0=padc[:], scalar1=-300.0,
                                scalar2=None, op0=OP.add)
        own = af("own", 1).bitcast(dt.int32)
        idxs = af("idx", S).bitcast(dt.int32)

        ald1 = cst.tile([P, NCOL * 8], dt.float32)
        ald2 = cst.tile([P, NCOL], dt.float32)

        with tc.tile_pool(name="slotp", bufs=1) as slotp:
            x1slot = slotp.tile([P, NCOL * 64], dt.float32)
            nc.scalar.memzero(x1slot[:])

            # ---------------- P1: own-shard L1 projection ----------------
            with tc.tile_pool(name="p1", bufs=3) as p1, \
                 tc.tile_pool(name="p1ps", bufs=4, space="PSUM") as p1ps:
                GT = 6
                t = 0
                while t < ncu:
                    g = min(GT, ncu - t)
                    xt = p1.tile([P, GT * P], dt.bfloat16, tag="xt")
                    nc.sync.dma_start(out=xt[:, :g * P],
                                      in_=ins["xT"][:, t * P:(t + g) * P])
                    ps = p1ps.tile([P, GT * R1F], dt.float32, tag="ps")
                    for i in range(g):
                        nc.tensor.matmul(out=ps[:, i * R1F:(i + 1) * R1F],
                                         lhsT=xt[:, i * P:(i + 1) * P],
                                         rhs=rhs1[:], start=True, stop=True)
                    st = p1.tile([P, GT * R1F], dt.bfloat16, tag="st")
                    nc.vector.tensor_copy(out=st[:, :g * R1F], in_=ps[:, :g * R1F])
                    stv = st[:, :g * R1F].rearrange("p (t r) -> p t r", r=R1F)
                    nc.sync.dma_start(
                        out=t1shard[:].rearrange("(p t) r -> p t r", p=P)[:, t:t + g],
                        in_=stv[:, :, 0:R1])
                    nc.sync.dma_start(
                        out=aldsh[:].rearrange("(p t) r -> p t r", p=P)[:, t:t + g],
                        in_=stv[:, :, R1:R1F])
                    t += g
                # zero the unused tail columns [ncu, 128)
                if ncu < NCOL:
                    zt = p1.tile([P, (NCOL - ncu) * R1], dt.bfloat16, tag="zt")
                    nc.scalar.memzero(zt[:])
                    nc.sync.dma_start(
                        out=t1shard[:].rearrange("(p t) r -> p (t r)", p=P)[:, ncu * R1:],
                        in_=zt[:])
            # pad row (local row 127 = slot (p=0, j=127), invalid by assert)
            nc.sync.dma_start(out=t1shard[PADROW:PADROW + 1, 64:72], in_=padc[0:1, :])

            # ---------------- AllGather table1 ----------------
            nc.gpsimd.collective_compute(
                "AllGather", mybir.AluOpType.bypass,
                replica_groups=[list(range(NCORES))],
                ins=[t1shard[:].opt()], outs=[table1[:].opt()])

            # al_d1 for own nodes: direct strided DMA from the local side table
            ald1b = cst.tile([P, NCOL * 8], dt.bfloat16)
            nc.sync.dma_start(
                out=ald1b[:, :ncu * 8],
                in_=aldsh[:].rearrange("(p t) r -> p (t r)", p=P)[:, :ncu * 8])
            nc.scalar.memzero(ald1[:])
            nc.vector.tensor_copy(out=ald1[:, :ncu * 8], in_=ald1b[:, :ncu * 8])

            # ---------------- P3: L1 edge phase ----------------
            if not os.environ.get("GAT_NOEDGE"):
                _edge_phase(tc, idxs, meta, layer=1, table=table1, ald=ald1,
                            out_slot=x1slot, wpool=None, pool_psum=None)

            # bias + relu
            nc.vector.tensor_tensor(
                out=x1slot[:].rearrange("p (n f) -> p n f", f=64),
                in0=x1slot[:].rearrange("p (n f) -> p n f", f=64),
                in1=af("b1", 64).rearrange("p (o f) -> p o f", o=1).to_broadcast([P, NCOL, 64]),
                op=OP.add)
            nc.scalar.activation(out=x1slot[:], in_=x1slot[:], func=ACTF.Relu)

            if os.environ.get("GAT_STOP"):
                nc.sync.dma_start(out=ins["out"][:, 0:1],
                                  in_=x1slot[0:gmax, 0:1])
                return

            # ------------- P4: transpose x1, AllGather x1 -------------
            with tc.tile_pool(name="p4", bufs=1) as p4:
                x1T = p4.tile([64, SHROWS], dt.bfloat16)
                with tc.tile_pool(name="p4ps", bufs=4, space="PSUM") as p4ps:
                    for j2 in range(0, NCOL, 2):
                        ps = p4ps.tile([64, 2 * P], dt.float32, tag="tp")
                        for k in range(2):
                            j = j2 + k
                            nc.tensor.transpose(
                                out=ps[:, k * P:(k + 1) * P],
                                in_=x1slot[:, j * 64:(j + 1) * 64], identity=ident[:])
                        nc.vector.tensor_copy(out=x1T[:, j2 * P:(j2 + 2) * P], in_=ps[:])
                nc.sync.dma_start(out=x1sh[:], in_=x1T[:])
        nc.gpsimd.collective_compute(
            "AllGather", mybir.AluOpType.bypass,
            replica_groups=[list(range(NCORES))],
            ins=[x1sh[:].opt()], outs=[x1full[:].opt()])

        # ---------------- P5: full local L2 projection ----------------
        t2v = table2[:].rearrange("(o p t) r -> p o (t r)", o=NCORES, p=P)
        with tc.tile_pool(name="p5", bufs=3) as p5, \
             tc.tile_pool(name="p5ps", bufs=4, space="PSUM") as p5ps:
            GL = 8
            GP = 3
            for o in range(NCORES):
                for jl in range(0, NCOL, GL):
                    blk = p5.tile([64, GL * P], dt.bfloat16, tag="blk")
                    nc.sync.dma_start(out=blk[:],
                                      in_=x1full[o * 64:(o + 1) * 64, jl * P:(jl + GL) * P])
                    jp = 0
                    while jp < GL:
                        gp = min(GP, GL - jp)
                        ps = p5ps.tile([P, GP * R2], dt.float32, tag="ps2")
                        for i in range(gp):
                            nc.tensor.matmul(
                                out=ps[:, i * R2:(i + 1) * R2],
                                lhsT=blk[:, (jp + i) * P:(jp + i + 1) * P],
                                rhs=rhs2[:], start=True, stop=True)
                        st = p5.tile([P, GP * R2], dt.bfloat16, tag="st2")
                        nc.vector.tensor_copy(out=st[:, :gp * R2], in_=ps[:, :gp * R2])
                        tt = jl + jp
                        nc.sync.dma_start(
                            out=(t2v[:, o:o + 1, tt * R2:(tt + gp) * R2]
                                 .rearrange("p o x -> p (o x)")),
                            in_=st[:, :gp * R2])
                        jp += gp
        nc.sync.dma_start(out=table2[PADROW:PADROW + 1, 128:129],
                          in_=padc[0:1, 0:1])

        # ---------------- P6: al_d2 for own nodes ----------------
        with tc.tile_pool(name="p6", bufs=1) as p6:
            tmp = p6.tile([P, NCOL * R2], dt.bfloat16)
            nc.gpsimd.indirect_dma_start(
                out=tmp[:], out_offset=None, in_=table2[:],
                in_offset=bass.IndirectOffsetOnAxis(ap=own[:, 0:1], axis=0))
            nc.vector.tensor_copy(
                out=ald2[:],
                in_=tmp[:].rearrange("p (n r) -> p n r", r=R2)[:, :, 129:130])

        if os.environ.get("GAT_STOP2"):
            nc.sync.dma_start(out=ins["out"][:, 0:1], in_=ald2[0:gmax, 0:1])
            return

        # ---------------- P7: L2 edge phase + pooling ----------------
        with tc.tile_pool(name="pool", bufs=1) as poolp, \
             tc.tile_pool(name="poolps", bufs=1, space="PSUM") as poolps:
            wpool = poolp.tile([P, NCOL * gmax], dt.float32)
            wpv = wpool[:].rearrange("p (n g) -> p n g", g=gmax)
            nc.vector.tensor_tensor(
                out=wpv,
                in0=(af("gid", NCOL).rearrange("p (n o) -> p n o", o=1)
                     .to_broadcast([P, NCOL, gmax])),
                in1=(af("iota", gmax).rearrange("p (o g) -> p o g", o=1)
                     .to_broadcast([P, NCOL, gmax])),
                op=OP.is_equal)
            nc.vector.tensor_tensor(
                out=wpv, in0=wpv,
                in1=(af("wval", NCOL).rearrange("p (n o) -> p n o", o=1)
                     .to_broadcast([P, NCOL, gmax])),
                op=OP.mult)

            pool_ps = poolps.tile([gmax, 128], dt.float32)
            _edge_phase(tc, idxs, meta, layer=2, table=table2, ald=ald2,
                        out_slot=None, wpool=wpool, pool_psum=pool_ps)

            # ---------------- P8: head ----------------
            pooled = poolp.tile([gmax, 128], dt.float32)
            nc.vector.tensor_copy(out=pooled[:], in_=pool_ps[:])
            nc.vector.tensor_tensor(out=pooled[:], in0=pooled[:],
                                    in1=af("b2g", 128, 0, gmax), op=OP.add)
            with tc.tile_pool(name="hps", bufs=1, space="PSUM") as hps:
                pT_ps = hps.tile([P, gmax], dt.float32)
                nc.tensor.transpose(out=pT_ps[:], in_=pooled[:],
                                    identity=ident[:gmax, :gmax])
                pT = poolp.tile([P, gmax], dt.float32)
                nc.vector.tensor_copy(out=pT[:], in_=pT_ps[:])
                lg_ps = hps.tile([gmax, NCLS], dt.float32)
                nc.tensor.matmul(out=lg_ps[:], lhsT=pT[:], rhs=af("fcw", NCLS),
                                 start=True, stop=True)
                lg = poolp.tile([gmax, NCLS], dt.float32)
                nc.vector.tensor_copy(out=lg[:], in_=lg_ps[:])
            nc.vector.tensor_tensor(out=lg[:], in0=lg[:],
                                    in1=af("fcb", NCLS, 0, gmax), op=OP.add)
            # log_softmax
            m = poolp.tile([gmax, 1], dt.float32)
            nc.vector.tensor_reduce(out=m[:], in_=lg[:], axis=AX.X, op=OP.max)
            nc.vector.tensor_scalar(out=lg[:], in0=lg[:], scalar1=m[:],
                                    scalar2=None, op0=OP.subtract)
            ex = poolp.tile([gmax, NCLS], dt.float32)
            nc.scalar.activation(out=ex[:], in_=lg[:], func=ACTF.Exp)
            ss = poolp.tile([gmax, 1], dt.float32)
            nc.vector.tensor_reduce(out=ss[:], in_=ex[:], axis=AX.X, op=OP.add)
            nc.scalar.activation(out=ss[:], in_=ss[:], func=ACTF.Ln)
            nc.vector.tensor_scalar(out=lg[:], in0=lg[:], scalar1=ss[:],
                                    scalar2=None, op0=OP.subtract)
            nc.sync.dma_start(out=ins["out"][:], in_=lg[:])


def _edge_phase(tc, idxs, meta, layer, table, ald, out_slot, wpool, pool_psum):
    import concourse.bass as bass
    import concourse.mybir as mybir

    nc = tc.nc
    dt = mybir.dt
    OP = mybir.AluOpType
    ACTF = mybir.ActivationFunctionType
    gmax = meta["gmax"]
    batches = meta["batches"]

    if layer == 1:
        R, F, H = 72, 64, 8
        HOFF = 64
    else:
        R, F, H = 130, 128, 1
        HOFF = 128
    C = F // H

    last = batches[-1]
    first = batches[0]

    with tc.tile_pool(name=f"ed{layer}", bufs=3 if layer == 1 else 2) as ep, \
         tc.tile_pool(name=f"eds{layer}", bufs=3) as eps:
        for (w, j0, ncols, ec0) in batches:
            Sb = ncols * w
            ed = ep.tile([P, SB * R], dt.bfloat16, tag="ed")
            for s in range(Sb):
                nc.gpsimd.indirect_dma_start(
                    out=ed[:, s * R:(s + 1) * R], out_offset=None, in_=table[:],
                    in_offset=bass.IndirectOffsetOnAxis(
                        ap=idxs[:, ec0 + s:ec0 + s + 1], axis=0))

            edk = ed[:, :Sb * R].rearrange("p (k n r) -> p k n r", k=w, r=R)
            eds_v = ed[:, :Sb * R].rearrange("p (s r) -> p s r", r=R)
            # e = al_s[src] + al_d[dst] (al_d identical across the w planes)
            et = eps.tile([P, SB * H], dt.float32, tag="et")
            etv = et[:, :Sb * H]
            nc.vector.tensor_copy(
                out=etv.rearrange("p (s h) -> p s h", h=H),
                in_=eds_v[:, :, HOFF:HOFF + H])
            aldv = (ald[:].rearrange("p (n h) -> p n h", h=H)[:, j0:j0 + ncols]
                    .rearrange("p (o n) h -> p o n h", o=1)
                    .to_broadcast([P, w, ncols, H]))
            nc.vector.tensor_tensor(
                out=etv.rearrange("p (k n h) -> p k n h", k=w, h=H),
                in0=etv.rearrange("p (k n h) -> p k n h", k=w, h=H),
                in1=aldv, op=OP.add)
            # exp(leaky_relu(e))  (leaky = max(x, 0.2x))
            lt = eps.tile([P, SB * H], dt.float32, tag="lt")
            nc.vector.tensor_scalar(out=lt[:, :Sb * H], in0=etv, scalar1=0.2,
                                    scalar2=None, op0=OP.mult)
            nc.vector.tensor_tensor(out=etv, in0=etv, in1=lt[:, :Sb * H], op=OP.max)
            nc.scalar.activation(out=etv, in_=etv, func=ACTF.Exp)
            # s[d] = sum_k exp : dense plane adds (f32)
            NH = ncols * H
            s = eps.tile([P, NB * 8], dt.float32, tag="s")
            sv = s[:, :NH]
            if w == 1:
                nc.vector.tensor_scalar(out=sv, in0=et[:, :NH], scalar1=1e-16,
                                        scalar2=None, op0=OP.add)
            else:
                nc.vector.tensor_tensor(out=sv, in0=et[:, 0:NH],
                                        in1=et[:, NH:2 * NH], op=OP.add)
                for k in range(2, w):
                    nc.vector.tensor_tensor(out=sv, in0=sv,
                                            in1=et[:, k * NH:(k + 1) * NH],
                                            op=OP.add)
                nc.vector.tensor_scalar(out=sv, in0=sv, scalar1=1e-16,
                                        scalar2=None, op0=OP.add)
            nc.vector.reciprocal(out=sv, in_=sv)
            # alpha in bf16 for the h multiply
            etb = eps.tile([P, SB * H], dt.bfloat16, tag="etb")
            nc.vector.tensor_copy(out=etb[:, :Sb * H], in_=etv)
            # WH = h[src] * alpha (in place, bf16)
            if H == 1:
                hview = eds_v[:, :, 0:F]
                exv = (etb[:, :Sb].rearrange("p (s o) -> p s o", o=1)
                       .to_broadcast([P, Sb, F]))
            else:
                hview = eds_v[:, :, 0:F].rearrange("p s (h c) -> p s h c", h=H)
                exv = (etb[:, :Sb * H].rearrange("p (s h o) -> p s h o", h=H, o=1)
                       .to_broadcast([P, Sb, H, C]))
            nc.vector.tensor_tensor(out=hview, in0=hview, in1=exv, op=OP.mult)

            # out[d] = (sum_k WH) / s[d] : pairwise bf16+bf16->f32 plane adds
            if layer == 1:
                ov = (out_slot[:].rearrange("p (n f) -> p n f", f=F)
                      [:, j0:j0 + ncols])
                x2b = None
            else:
                x2b = ep.tile([P, NB * F], dt.float32, tag="x2b")
                ov = x2b[:, :ncols * F].rearrange("p (n f) -> p n f", f=F)

            def plane(k):
                return (edk[:, k:k + 1, :, 0:F]
                        .rearrange("p o n r -> p (o n) r"))
            acc = eps.tile([P, NB * F], dt.float32, tag="acc")
            av = acc[:, :ncols * F].rearrange("p (n f) -> p n f", f=F)
            if w == 1:
                nc.vector.tensor_copy(out=ov, in_=plane(0))
            else:
                nc.vector.tensor_tensor(out=ov, in0=plane(0), in1=plane(1),
                                        op=OP.add)
                k = 2
                while k + 1 < w:
                    nc.vector.tensor_tensor(out=av, in0=plane(k),
                                            in1=plane(k + 1), op=OP.add)
                    nc.vector.tensor_tensor(out=ov, in0=ov, in1=av, op=OP.add)
                    k += 2
                if k < w:
                    nc.vector.tensor_copy(out=av, in_=plane(k))
                    nc.vector.tensor_tensor(out=ov, in0=ov, in1=av, op=OP.add)
            if H == 1:
                sinvv = (sv.rearrange("p (n o) -> p n o", o=1)
                         .to_broadcast([P, ncols, F]))
                ovv = ov
            else:
                sinvv = (sv.rearrange("p (n h o) -> p n h o", h=H, o=1)
                         .to_broadcast([P, ncols, H, C]))
                ovv = ov.rearrange("p n (h c) -> p n h c", h=H)
            nc.vector.tensor_tensor(out=ovv, in0=ovv, in1=sinvv, op=OP.mult)

            if layer == 2:
                for jj in range(ncols):
                    nc.tensor.matmul(
                        out=pool_psum[:],
                        lhsT=wpool[:, (j0 + jj) * gmax:(j0 + jj + 1) * gmax],
                        rhs=x2b[:, jj * F:(jj + 1) * F],
                        start=((w, j0, ncols, ec0) == first and jj == 0),
                        stop=((w, j0, ncols, ec0) == last and jj == ncols - 1),
                        skip_group_check=True)


# ----------------------------------------------------------------------------
# runner
# ----------------------------------------------------------------------------

_CACHE = {}


def _get_nc(meta, in_map0):
    key = str(sorted(meta.items(), key=lambda kv: kv[0]))
    if key in _CACHE:
        return _CACHE[key]
    import concourse.bacc as bacc
    import concourse.tile as tile
    import concourse.mybir as mybir
    dt = mybir.dt
    nc = bacc.Bacc("TRN2", target_bir_lowering=False, debug=False,
                   num_devices=NCORES)
    ins = {}
    for name, arr in in_map0.items():
        ins[name] = nc.dram_tensor(name, list(arr.shape),
                                   _np_dtype_to_bir(arr),
                                   kind="ExternalInput").ap()
    ins["out"] = nc.dram_tensor("out", [meta["gmax"], NCLS], dt.float32,
                                kind="ExternalOutput").ap()
    with tile.TileContext(nc) as tc:
        build_program(tc, ins, meta)
    nc.compile()
    _CACHE[key] = nc
    return nc


def _np_dtype_to_bir(a):
    import concourse.mybir as mybir
    import ml_dtypes
    dt = mybir.dt
    if a.dtype == np.int32:
        return dt.int32
    if a.dtype == ml_dtypes.bfloat16:
        return dt.bfloat16
    return dt.float32


def make_inputs(x, edge_index, batch, W1, a_src1, a_dst1, b1, W2, a_src2,
                a_dst2, b2, fc_w, fc_b):
    import ml_dtypes
    x = np.asarray(x, np.float32)
    host, meta, auxd = _prep(x, np.asarray(edge_index), np.asarray(batch))
    gmax = meta["gmax"]
    ncu = meta["ncu"]
    lay = _aux_layout(meta)
    n0 = meta["n0"]

    ge = np.searchsorted(np.asarray(batch), np.arange(NG), side="left")
    gEnd = np.searchsorted(np.asarray(batch), np.arange(NG), side="right")

    in_maps = []
    for c in range(NCORES):
        h = host[c]
        aux = np.zeros((P, lay["_total"]), np.float32)

        def put(name, arr, p0=0):
            arr = np.asarray(arr, np.float32)
            aux[p0:p0 + arr.shape[0], lay[name]:lay[name] + arr.shape[1]] = arr

        put("gid", h["gid"])
        put("wval", h["wval"])
        put("iota", np.tile(np.arange(gmax, dtype=np.float32).reshape(1, gmax),
                            (P, 1)))
        aux[:, lay["own"]:lay["own"] + 1] = h["ownbase"].view(np.float32)
        g0c, g1c = meta["g0"][c], meta["g1"][c]
        nonempty = np.zeros((gmax, 1), np.float32)
        cntc = (gEnd - ge)[g0c:g1c]
        nonempty[:g1c - g0c, 0] = (cntc > 0).astype(np.float32)
        put("b2g", nonempty * np.asarray(b2, np.float32).reshape(1, 128))
        put("fcb", np.tile(np.asarray(fc_b, np.float32).reshape(1, NCLS),
                           (gmax, 1)))
        put("W1", np.asarray(W1, np.float32))
        put("a1s", np.tile(np.asarray(a_src1, np.float32).reshape(1, 64), (P, 1)))
        put("a1d", np.tile(np.asarray(a_dst1, np.float32).reshape(1, 64), (P, 1)))
        put("b1", np.tile(np.asarray(b1, np.float32).reshape(1, 64), (P, 1)))
        put("W2", np.asarray(W2, np.float32))
        put("a2s", np.tile(np.asarray(a_src2, np.float32).reshape(1, 128), (64, 1)))
        put("a2d", np.tile(np.asarray(a_dst2, np.float32).reshape(1, 128), (64, 1)))
        put("fcw", np.asarray(fc_w, np.float32))
        aux[:, lay["idx"]:lay["idx"] + meta["S"]] = h["idx"].view(np.float32)

        # x permuted into slot order, bf16, [feat, slot], used columns only
        sn = auxd["slot_nodes"][c]
        valid = sn >= 0
        xs = np.zeros((ncu * P, x.shape[1]), np.float32)
        ppi, jji = np.nonzero(valid)
        xs[jji * P + ppi] = x[n0[c] + sn[ppi, jji]]
        xT = np.ascontiguousarray(xs.T).astype(ml_dtypes.bfloat16)

        in_maps.append(dict(xT=xT, aux=aux))
    return in_maps, meta, auxd


def kernel(x, edge_index, batch, W1, a_src1, a_dst1, b1, W2, a_src2, a_dst2,
           b2, fc_w, fc_b):
    in_maps, meta, auxd = make_inputs(x, edge_index, batch, W1, a_src1, a_dst1,
                                      b1, W2, a_src2, a_dst2, b2, fc_w, fc_b)
    global _LAST
    _LAST = dict(meta=meta, aux=auxd)
    nc = _get_nc(meta, in_maps[0])
    from concourse.bass_utils import run_bass_kernel_spmd
    res = run_bass_kernel_spmd(nc, in_maps, core_ids=list(range(NCORES)))
    _LAST["res"] = res
    out = np.zeros((NG, NCLS), np.float32)
    for c in range(NCORES):
        g0, g1 = meta["g0"][c], meta["g1"][c]
        out[g0:g1] = res.results[c]["out"][:g1 - g0]
    return out
